# revision 1
# baseline (speedup 1.0000x reference)
"""Average Hausdorff loss on 8 Trainium2 NeuronCores.

Strategy
--------
Host (numpy, cheap): binarize masks, 3x3-erosion edge detection, compact
edge-pixel coordinates per (b, c) pair, build "augmented" coordinate
matrices so that a single K=6 bf16 matmul on the PE array produces the
exact value  -(squared distance)/4  for a [128 gth-pts, N pred-pts] tile
in PSUM (all products/partial sums are integers*0.25 < 2^24 -> exact
fp32; coords are centered so byte-split squared norms fit bf16 exactly).

Device (raw Bass, SPMD over 8 cores, 2 (b,c) pairs per core), pipelined
over PE -> ACT -> DVE per [128 gth x 1536 pred] chunk:
  PE : 3 matmuls -> PSUM = -(d^2)/4
  ACT: activation Copy with scale 2^-12 -> SBUF fp16 (sole PSUM reader)
  DVE: two fp16 2x halving folds + short reduce-max -> gth->pred NN,
       one fp16 2x tensor_max accumulate -> pred->gth NN
Host: final partition reduce for the pred->gth direction, sqrt, masked
means, nanmean -- tiny.

Pad points use a far sentinel coordinate so they never win a max.
"""

import numpy as np

H = 256
W = 256
BC = 16          # B*C pairs
N_CORES = 8
PAIRS_PER_CORE = 2
P_CHUNK = 1536   # pred points per DVE op (3 PSUM banks)
G_TILE = 128     # gth points per PE tile (PSUM partitions)
SENT = 16384.0   # sentinel coordinate (centered space), 2^14
D2_SCALE = 2.0 ** -12   # extra scale on -(d^2)/4 so fp16 never overflows
D2_BACK = -4.0 * 4096.0  # value -> d^2


def _edge_maps(x):
    """[BC, H, W] float -> bool edge maps, matching the reference:
    edge = mask & ~erode3x3(mask), erosion padded with True."""
    m = x > 0.5
    p = np.pad(m, ((0, 0), (1, 1), (1, 1)), constant_values=True)
    e = np.ones_like(m)
    for dy in range(3):
        for dx in range(3):
            e &= p[:, dy:dy + H, dx:dx + W]
    return m & ~e


def _compact_coords(edge):
    """bool [H, W] -> (cy, cx) float32 arrays of centered coords."""
    ys, xs = np.nonzero(edge)
    return (ys.astype(np.float32) - 128.0), (xs.astype(np.float32) - 128.0)


def _aug_g(cy, cx, n_pad):
    """lhsT rows [6, n_pad] for the stationary (gth) operand."""
    n = cy.shape[0]
    out = np.zeros((6, n_pad), np.float32)
    fy = np.full(n_pad, SENT, np.float32)
    fx = np.full(n_pad, SENT, np.float32)
    fy[:n] = cy
    fx[:n] = cx
    sq = fy * fy + fx * fx
    b1 = np.floor(sq / 256.0)
    b0 = sq - b1 * 256.0
    out[0] = fy * 0.5
    out[1] = fx * 0.5
    out[2] = -b1
    out[3] = -b0
    out[4] = -64.0
    out[5] = -0.25
    return out


def _aug_p(cy, cx, n_pad):
    """rhs rows [6, n_pad] for the moving (pred) operand."""
    n = cy.shape[0]
    out = np.zeros((6, n_pad), np.float32)
    fy = np.full(n_pad, SENT, np.float32)
    fx = np.full(n_pad, SENT, np.float32)
    fy[:n] = cy
    fx[:n] = cx
    sq = fy * fy + fx * fx
    b1 = np.floor(sq / 256.0)
    b0 = sq - b1 * 256.0
    out[0] = fy
    out[1] = fx
    out[2] = 64.0
    out[3] = 0.25
    out[4] = b1
    out[5] = b0
    return out


def _build_program(structure, self_waits=False):
    """structure: tuple of (n_gtiles, n_pchunks) per pair slot.

    Raw-bass program (no Tile): explicit semaphores, standalone waits.
    This walrus build rejects matmuls carrying >1 inline sync-wait, so
    the streams are arranged such that every instruction needs at most
    one cross-engine wait, emitted as its own EventSemaphore.

    self_waits adds same-engine DVE waits for RAW/WAR chains. Hardware
    orders these via the engine FIFO + per-op pipeline drain; the waits
    exist only to satisfy CoreSim's race detector (sim builds).
    """
    from contextlib import ExitStack
    import concourse.bass as bass
    import concourse.mybir as mybir

    f32 = mybir.dt.float32
    f16 = mybir.dt.float16
    bf16 = mybir.dt.bfloat16
    MAX = mybir.AluOpType.max

    nc = bass.Bass()

    gaug_d, paug_d, dg_d, dp_d = [], [], [], []
    for s, (tg, npc) in enumerate(structure):
        ng_pad = tg * G_TILE
        np_pad = npc * P_CHUNK
        gaug_d.append(nc.declare_dram_parameter(f"gaug{s}", [6, ng_pad], bf16,
                                                isOutput=False))
        paug_d.append(nc.declare_dram_parameter(f"paug{s}", [6, np_pad], bf16,
                                                isOutput=False))
        dg_d.append(nc.declare_dram_parameter(f"dg{s}", [G_TILE, tg], f32,
                                              isOutput=True))
        dp_d.append(nc.declare_dram_parameter(f"dp{s}", [G_TILE, np_pad], f16,
                                              isOutput=True))

    n_slots = len(structure)
    total_chunks = sum(tg * npc for tg, npc in structure)
    NB = 4  # d2s fp16 ring depth

    with ExitStack() as ctx:
        gs, ps, dp_acc, dg_st, dg_all = [], [], [], [], []
        for s, (tg, npc) in enumerate(structure):
            gs.append(ctx.enter_context(
                nc.sbuf_tensor(f"gs{s}", [6, tg * G_TILE], bf16)))
            ps.append(ctx.enter_context(
                nc.sbuf_tensor(f"ps{s}", [6, npc * P_CHUNK], bf16)))
            dp_acc.append(ctx.enter_context(
                nc.sbuf_tensor(f"dpacc{s}", [G_TILE, npc * P_CHUNK], f16)))
            dg_st.append(ctx.enter_context(
                nc.sbuf_tensor(f"dgst{s}", [G_TILE, tg, npc], f32)))
            dg_all.append(ctx.enter_context(
                nc.sbuf_tensor(f"dgall{s}", [G_TILE, tg], f32)))
        pt = [ctx.enter_context(nc.psum_tensor(f"pt{i}", [G_TILE, P_CHUNK], f32))
              for i in range(2)]
        # fp16 distance ring: 4 chunk slots in one tensor so adjacent pairs
        # (even k, odd k) can be consumed by single wide DVE ops.
        d2s = ctx.enter_context(
            nc.sbuf_tensor("d2s", [G_TILE, NB, P_CHUNK], f16))
        # fold buffers for the dg reduction (fp16 tt_max halving steps)
        fd1 = [ctx.enter_context(
            nc.sbuf_tensor(f"fd1_{i}", [G_TILE, 2, P_CHUNK // 2], f16))
            for i in range(2)]
        fd2 = [ctx.enter_context(
            nc.sbuf_tensor(f"fd2_{i}", [G_TILE, 2, P_CHUNK // 4], f16))
            for i in range(2)]
        fd3 = [ctx.enter_context(
            nc.sbuf_tensor(f"fd3_{i}", [G_TILE, P_CHUNK // 4], f16))
            for i in range(2)]
        fd4 = [ctx.enter_context(
            nc.sbuf_tensor(f"fd4_{i}", [G_TILE, P_CHUNK // 8], f16))
            for i in range(2)]

        dma_sems = [ctx.enter_context(nc.semaphore(f"dma_in{s}"))
                    for s in range(n_slots)]
        pe_sem = ctx.enter_context(nc.semaphore("pe_done"))
        act_sem = ctx.enter_context(nc.semaphore("act_done"))
        dve_sem = ctx.enter_context(nc.semaphore("dve_done"))
        out_sem = ctx.enter_context(nc.semaphore("dma_out"))
        block = ctx.enter_context(nc.Block())

        # Dry run of the DVE emission to get exact dve_sem values.
        # Groups: one per (slot, gt). npc==2 groups use paired (3072-wide)
        # DVE ops; other npc use per-chunk ops. 4 DVE incs per chunk-pair /
        # per chunk respectively; +1 final dg reduce per slot.
        chunk_last_read = []   # per chunk k: dve_sem when its d2s reads done
        slot_end = []
        _n = 0
        _k = 0
        for tg, npc in structure:
            paired = (npc == 2 and _k % 2 == 0)
            for gt in range(tg):
                if paired:
                    # flat group: 4 folds + reduce + dp max = 6 ops
                    _n += 6
                    chunk_last_read += [_n, _n]
                    _k += 2
                else:
                    for _ in range(npc):
                        _n += 4
                        chunk_last_read.append(_n)
                        _k += 1
            if not paired:
                _n += 1  # slot-final dg reduce (fallback path only)
            slot_end.append(_n)

        @block.sync
        def _(sync):
            for s in range(n_slots):
                sync.dma_start(gs[s][:], gaug_d[s][:]).then_inc(dma_sems[s], 16)
                sync.dma_start(ps[s][:], paug_d[s][:]).then_inc(dma_sems[s], 16)
            for s in range(n_slots):
                sync.wait_ge(dve_sem, slot_end[s])
                sync.dma_start(dg_d[s][:], dg_all[s][:]).then_inc(out_sem, 16)
                sync.dma_start(dp_d[s][:], dp_acc[s][:]).then_inc(out_sem, 16)
            # No final out_sem wait: the block-end drain waits the DMA
            # HW queues, so output completion is already guaranteed.

        @block.tensor
        def _(tensor):
            k = 0
            for s, (tg, npc) in enumerate(structure):
                # start as soon as THIS slot's inputs have landed
                tensor.wait_ge(dma_sems[s], 32)
                for gt in range(tg):
                    lhsT = gs[s][:, gt * G_TILE:(gt + 1) * G_TILE]
                    for pc in range(npc):
                        if k >= 2:
                            # psum slot reuse: ACT (sole PSUM reader) of
                            # chunk k-2 done
                            tensor.wait_ge(act_sem, k - 1)
                        p = pt[k % 2]
                        for b in range(P_CHUNK // 512):
                            off = pc * P_CHUNK + b * 512
                            mm = nc.tensor.matmul(
                                p[:, b * 512:(b + 1) * 512],
                                lhsT,
                                ps[s][:, off:off + 512],
                                start=True, stop=True,
                            )
                        mm.then_inc(pe_sem, 1)
                        k += 1

        @block.scalar
        def _(scalar):
            # PSUM fp32 -> SBUF fp16, scaled by 2^-12 so sentinel-pad
            # distances stay finite in fp16 (power-of-2: real values
            # keep their mantissa exactly).
            for k in range(total_chunks):
                scalar.wait_ge(pe_sem, k + 1)
                if k >= NB:
                    scalar.wait_ge(dve_sem, chunk_last_read[k - NB])
                nc.scalar.activation(
                    d2s[:, k % NB, :], pt[k % 2][:],
                    mybir.ActivationFunctionType.Copy, scale=D2_SCALE,
                ).then_inc(act_sem, 1)

        @block.vector
        def _(vector):
            H1 = P_CHUNK // 2
            H2 = P_CHUNK // 4
            k = 0
            n_ops = 0
            gi = 0            # group (gt) counter, for fold ring indexing
            writer = {}       # dp_acc region -> op count of its last write
            f_free = {}       # fold ring slot -> op count after its last read

            def dg_fold(din0, din1, f1, f1a, f1b, f2, out_col, ring):
                """fold-fold-reduce: d halves -> f1 -> f2 -> reduce."""
                nonlocal n_ops
                w = f_free.get(("f1", ring))
                if self_waits and w:
                    vector.wait_ge(dve_sem, w)  # f1 ring WAR
                nc.vector.tensor_max(f1, din0, din1).then_inc(dve_sem, 1)
                n_ops += 1
                w = f_free.get(("f2", ring))
                if self_waits:
                    vector.wait_ge(dve_sem, max(n_ops, w or 0))
                nc.vector.tensor_max(f2, f1a, f1b).then_inc(dve_sem, 1)
                n_ops += 1
                f_free[("f1", ring)] = n_ops
                if self_waits:
                    vector.wait_ge(dve_sem, n_ops)  # f2 RAW
                nc.vector.tensor_reduce(
                    out_col, f2, axis=mybir.AxisListType.X, op=MAX,
                ).then_inc(dve_sem, 1)
                n_ops += 1
                f_free[("f2", ring)] = n_ops

            def dp_accum(dpc, src, first):
                nonlocal n_ops
                if first:
                    ins = nc.vector.tensor_copy(dpc, src)
                else:
                    if self_waits:
                        vector.wait_ge(dve_sem, writer[id(dpc.tensor)])
                    ins = nc.vector.tensor_max(dpc, dpc, src)
                ins.then_inc(dve_sem, 1)
                n_ops += 1

            for s, (tg, npc) in enumerate(structure):
                paired = (npc == 2 and k % 2 == 0)
                for gt in range(tg):
                    r = gi % 2
                    if paired:
                        pr = k % NB  # even, pair occupies slots pr, pr+1
                        vector.wait_ge(act_sem, k + 2)
                        dpair = d2s[:, pr:pr + 2, :].rearrange("p a b -> p (a b)")
                        # flat fold chain over the whole 3072-wide group:
                        # each step halves at fp16 2x; tiny 1x reduce last.
                        chain = [
                            fd1[r][:].rearrange("p a b -> p (a b)"),
                            fd2[r][:].rearrange("p a b -> p (a b)"),
                            fd3[r][:],
                            fd4[r][:],
                        ]
                        src = dpair
                        W = 2 * P_CHUNK
                        for buf in chain:
                            if self_waits:
                                vector.wait_ge(dve_sem, n_ops)
                            nc.vector.tensor_max(
                                buf[:, 0:W // 2],
                                src[:, 0:W // 2], src[:, W // 2:W],
                            ).then_inc(dve_sem, 1)
                            n_ops += 1
                            src = buf
                            W //= 2
                        if self_waits:
                            vector.wait_ge(dve_sem, n_ops)
                        nc.vector.tensor_reduce(
                            dg_all[s][:, gt:gt + 1], src[:, 0:W],
                            axis=mybir.AxisListType.X, op=MAX,
                        ).then_inc(dve_sem, 1)
                        n_ops += 1
                        dpc = dp_acc[s][:, 0:2 * P_CHUNK]
                        dp_accum(dpc, dpair, gt == 0)
                        writer[id(dpc.tensor)] = n_ops
                        k += 2
                    else:
                        for pc in range(npc):
                            vector.wait_ge(act_sem, k + 1)
                            c = k % NB
                            f1 = fd1[r][:, 0, :]
                            f2 = fd2[r][:, 0, :]
                            dg_fold(
                                d2s[:, c, 0:H1], d2s[:, c, H1:P_CHUNK],
                                f1, f1[:, 0:H2], f1[:, H2:H1],
                                f2, dg_st[s][:, gt, pc:pc + 1], r,
                            )
                            dpc = dp_acc[s][:, pc * P_CHUNK:(pc + 1) * P_CHUNK]
                            dp_accum(dpc, d2s[:, c, :], gt == 0)
                            writer[id(dpc.tensor)] = n_ops
                            k += 1
                    gi += 1
                if not paired:
                    if self_waits:
                        vector.wait_ge(dve_sem, n_ops)  # dg_st writes done
                    nc.vector.tensor_reduce(
                        dg_all[s][:], dg_st[s][:],
                        axis=mybir.AxisListType.X, op=MAX,
                    ).then_inc(dve_sem, 1)
                    n_ops += 1

    return nc


def _loss_from_nn(dg_val, dp_val, n_g, n_p):
    """Mirror the reference combination. dg_val/dp_val are the device maxes
    of -(d^2)/4 * 2^-12 for the first n_g / n_p (valid) points."""
    with np.errstate(divide="ignore", invalid="ignore", over="ignore"):
        d_g = np.sqrt(np.maximum(D2_BACK * dg_val.astype(np.float64), 0.0))
        d_p = np.sqrt(np.maximum(D2_BACK * dp_val.astype(np.float64), 0.0))
        gth2pred = d_g.sum() / n_g if n_g > 0 else np.float64(np.nan)
        pred2gth = d_p.sum() / n_p if n_p > 0 else np.float64(np.nan)
        ahd = (gth2pred + pred2gth) / 2.0
        if n_g == 0 and n_p == 0:
            ahd = np.float64(np.nan)
        return 1.0 - 1.0 / (1.0 + ahd)


RUN_OPTS = {}    # extra kwargs for run_bass_kernel_spmd (test harness hook)
LAST_RES = None  # last BassKernelResults (test harness hook)


def kernel(gth, pred):
    from concourse.bass_utils import run_bass_kernel_spmd
    import ml_dtypes

    gth = np.asarray(gth, np.float32).reshape(BC, H, W)
    pred = np.asarray(pred, np.float32).reshape(BC, H, W)

    gedge = _edge_maps(gth)
    pedge = _edge_maps(pred)
    pts = []
    for i in range(BC):
        gy, gx = _compact_coords(gedge[i])
        py, px = _compact_coords(pedge[i])
        pts.append((gy, gx, py, px))

    # Balance pairs across cores: sort by tile cost, big+small per core.
    def cost(i):
        gy = pts[i][0]
        py = pts[i][2]
        return (max(1, -(-len(gy) // G_TILE)) * max(1, -(-len(py) // P_CHUNK)))
    order = sorted(range(BC), key=cost, reverse=True)
    assign = [[order[c], order[BC - 1 - c]] for c in range(N_CORES)]

    # Uniform per-slot structure = max over cores.
    structure = []
    for s in range(PAIRS_PER_CORE):
        tg = max(max(1, -(-len(pts[assign[c][s]][0]) // G_TILE))
                 for c in range(N_CORES))
        npc = max(max(1, -(-len(pts[assign[c][s]][2]) // P_CHUNK))
                  for c in range(N_CORES))
        structure.append((tg, npc))
    structure = tuple(structure)

    nc = _build_program(structure)

    in_maps = []
    for c in range(N_CORES):
        m = {}
        for s in range(PAIRS_PER_CORE):
            tg, npc = structure[s]
            gy, gx, py, px = pts[assign[c][s]]
            m[f"gaug{s}"] = _aug_g(gy, gx, tg * G_TILE).astype(ml_dtypes.bfloat16)
            m[f"paug{s}"] = _aug_p(py, px, npc * P_CHUNK).astype(ml_dtypes.bfloat16)
        in_maps.append(m)

    res = run_bass_kernel_spmd(nc, in_maps, list(range(N_CORES)), **RUN_OPTS)
    global LAST_RES
    LAST_RES = res
    results = res.results

    losses = np.full(BC, np.nan, np.float64)
    for c in range(N_CORES):
        for s in range(PAIRS_PER_CORE):
            i = assign[c][s]
            gy, gx, py, px = pts[i]
            n_g, n_p = len(gy), len(py)
            dg = np.asarray(results[c][f"dg{s}"], np.float64)   # [128, tg]
            dp = np.asarray(results[c][f"dp{s}"], np.float64)   # [128, np_pad]
            dg_flat = dg.T.reshape(-1)[:n_g]
            dp_red = dp.max(axis=0)[:n_p]
            losses[i] = _loss_from_nn(dg_flat, dp_red, n_g, n_p)

    return np.float32(np.nanmean(losses.astype(np.float32)))



# revision 13
# speedup vs baseline: 3.1085x; 3.1085x over previous
"""Average Hausdorff loss on 8 Trainium2 NeuronCores.

Strategy (v2: pruned two-direction NN, grouped multi-engine reductions)
----------------------------------------------------------------------
Host (numpy, cheap):
  * binarize + 3x3-erosion edge detection, compact edge coords per (b,c)
  * per direction (g->p and p->g): KD-split the query points into tiles
    of <=128; a cell-grid separable EDT gives a per-point upper bound on
    the NN distance; each tile's candidate set = DB points within the
    per-16-query-subblock bbox expanded by the subblock's max bound.
    The candidate set provably contains every query's true NN.
  * each (tile, candidates) job is cut into uniform 256-candidate
    PIECES. Pieces are bin-packed across the 8 cores; the device
    program is piece-index uniform (all per-core variation lives in the
    DMAed data: each piece has its own stationary copy + moving block).

Device (raw Bass, SPMD over 8 cores):
  PE   : per piece, one matmul [6,128]^T @ [6,256] -> PSUM -(d^2)/4
         (exact in bf16 via byte-split squared norms).
  Pieces are consumed in GROUPS by one of three statically assigned
  reduce paths (PSUM is split into 3 double-buffered regions):
    D: DVE tensor_reduce [128,T,256] -> [128,T] f32 straight from PSUM
    A: ACT copies the group to SBUF fp16 (scale 2^-12), DVE finishes
       with a 2x/4x fp16 tensor_reduce
    G: GPSIMD copies the group to SBUF fp16 via tensor_scalar
       ((x min 0) * 2^-12), DVE finishes with the fp16 tensor_reduce
  Each piece yields one temp column (per-query max of -(d^2)/4).
Host: per job take max over its pieces' columns -> NN distance per
query point; sqrt, masked means, nanmean.
"""

import math
import numpy as np

H = 256
W_IMG = 256
BC = 16
N_CORES = 8
TILE_Q = 128          # query points per job
SUB_Q = 16            # sub-block size for candidate bbox union
CELL = 2              # bound-grid cell size in px
WP = 256              # uniform piece width (candidate cols)
SENT = 16384.0        # sentinel coordinate (centered space)
D2_SCALE = 2.0 ** -12
ACC_INIT = -1.0e30
PATHS = "DA"          # enabled reduce paths (subset of "DAG")

# PSUM regions: (start_col, n_group_slots=2, group_cols)
REG = {
    "D": (0, 512),
    "A": (1024, 1024),
    "G": (3072, 512),
}


def _edge_maps(x):
    """[BC, H, W] float -> bool edge maps (mask & ~erode3x3, pad True)."""
    m = x > 0.5
    p = np.pad(m, ((0, 0), (1, 1), (1, 1)), constant_values=True)
    e = np.ones_like(m)
    for dy in range(3):
        for dx in range(3):
            e &= p[:, dy:dy + H, dx:dx + W_IMG]
    return m & ~e


def _aug_g(cy, cx, n_pad):
    """Stationary operand rows [6, n_pad] (query side)."""
    n = cy.shape[0]
    fy = np.full(n_pad, SENT, np.float32)
    fx = np.full(n_pad, SENT, np.float32)
    fy[:n] = cy
    fx[:n] = cx
    sq = fy * fy + fx * fx
    b1 = np.floor(sq / 256.0)
    b0 = sq - b1 * 256.0
    out = np.empty((6, n_pad), np.float32)
    out[0] = fy * 0.5
    out[1] = fx * 0.5
    out[2] = -b1
    out[3] = -b0
    out[4] = -64.0
    out[5] = -0.25
    return out


def _aug_p(cy, cx, n_pad):
    """Moving operand rows [6, n_pad] (candidate side)."""
    n = cy.shape[0]
    fy = np.full(n_pad, SENT, np.float32)
    fx = np.full(n_pad, SENT, np.float32)
    fy[:n] = cy
    fx[:n] = cx
    sq = fy * fy + fx * fx
    b1 = np.floor(sq / 256.0)
    b0 = sq - b1 * 256.0
    out = np.empty((6, n_pad), np.float32)
    out[0] = fy
    out[1] = fx
    out[2] = 64.0
    out[3] = 0.25
    out[4] = b1
    out[5] = b0
    return out


def _kd_tiles(ys, xs, tile):
    """Recursive median split into spatially compact blocks of <= tile pts."""
    out = []

    def rec(ix):
        if len(ix) <= tile:
            out.append(ix)
            return
        yy, xx = ys[ix], xs[ix]
        k = yy if (yy.max() - yy.min() >= xx.max() - xx.min()) else xx
        n = len(ix)
        half = (n // 2 // tile) * tile or n // 2
        o = np.argsort(k, kind="stable")
        rec(ix[o[:half]])
        rec(ix[o[half:]])

    rec(np.arange(len(ys)))
    return out


def _cell_ub(dys, dxs):
    """Per-cell upper bound on distance to the nearest DB point."""
    G = 256 // CELL
    occ = np.zeros((G, G), bool)
    occ[dys // CELL, dxs // CELL] = True
    BIG = np.int64(10 ** 9)
    ar = np.arange(G)
    d2 = (ar[:, None] - ar[None, :]) ** 2
    occf = np.where(occ, 0, BIG)
    gcol = (d2[:, :, None] + occf[None, :, :]).min(axis=1)
    D2 = (gcol[:, None, :] + d2[None, :, :]).min(axis=2)
    return np.sqrt(D2.astype(np.float64)) * CELL + math.sqrt(2.0) * CELL


def _build_jobs(qys, qxs, dys, dxs):
    """One direction of one pair -> list of (q_idx, cand_idx) jobs."""
    ub = _cell_ub(dys, dxs)[qys // CELL, qxs // CELL]
    jobs = []
    for ix in _kd_tiles(qys, qxs, TILE_Q):
        m = np.zeros(len(dys), bool)
        for s in range(0, len(ix), SUB_Q):
            sx = ix[s:s + SUB_Q]
            u = ub[sx].max()
            y0, y1 = qys[sx].min() - u, qys[sx].max() + u
            x0, x1 = qxs[sx].min() - u, qxs[sx].max() + u
            m |= (dys >= y0) & (dys <= y1) & (dxs >= x0) & (dxs <= x1)
        jobs.append((ix, np.nonzero(m)[0]))
    return jobs


def _plan_groups(n_pieces):
    """Assign consecutive pieces to path groups, balancing engine busy.

    Returns list of (path, piece_lo, piece_hi).
    """
    busy = {"DVE": 0.0, "ACT": 0.0, "GPS": 0.0}
    groups = []
    p = 0
    while p < n_pieces:
        cand = {}
        if "D" in PATHS:
            t = min(REG["D"][1] // WP, n_pieces - p)
            c = t * WP * 1.0417 + 125
            cand["D"] = (max(busy["DVE"] + c, 0), t, ("DVE", c))
        if "A" in PATHS:
            t = min(REG["A"][1] // WP, n_pieces - p)
            ca = t * WP * 0.833 + 143
            cd = t * WP * 0.52 + 60
            m = max(busy["ACT"] + ca, busy["DVE"] + cd)
            cand["A"] = (m, t, ("ACT", ca, "DVE", cd))
        if "G" in PATHS:
            t = min(REG["G"][1] // WP, n_pieces - p)
            cg = t * WP * 1.39 + 95
            cd = t * WP * 0.52 + 60
            m = max(busy["GPS"] + cg, busy["DVE"] + cd)
            cand["G"] = (m, t, ("GPS", cg, "DVE", cd))
        # pick path minimizing resulting max busy, normalized per piece
        best, bt = None, None
        for k, (m, t, upd) in cand.items():
            score = m / t
            if best is None or score < bt:
                best, bt = k, score
        m, t, upd = cand[best]
        for i in range(0, len(upd), 2):
            busy[upd[i]] += upd[i + 1]
        groups.append((best, p, p + t))
        p += t
    return groups


def _build_program(n_pieces, mov_chunks):
    """Raw-bass SPMD program, uniform over piece index."""
    from contextlib import ExitStack
    import concourse.bass as bass
    import concourse.mybir as mybir

    f32 = mybir.dt.float32
    f16 = mybir.dt.float16
    bf16 = mybir.dt.bfloat16
    MAX = mybir.AluOpType.max
    X = mybir.AxisListType.X

    groups = _plan_groups(n_pieces)

    nc = bass.Bass()
    stat_d = nc.declare_dram_parameter("stat", [6, 128 * n_pieces], bf16,
                                       isOutput=False)
    mov_d = nc.declare_dram_parameter("mov", [6, WP * n_pieces], bf16,
                                      isOutput=False)
    tmp_d = nc.declare_dram_parameter("tmp", [128, n_pieces], f32,
                                      isOutput=True)
    tmpa_d = nc.declare_dram_parameter("tmpa", [128, n_pieces], f16,
                                       isOutput=True)

    n_a = sum(1 for g in groups if g[0] == "A")
    n_d = sum(1 for g in groups if g[0] == "D")
    n_g = sum(1 for g in groups if g[0] == "G")

    # cumulative matmul count at end of each group == piece_hi
    with ExitStack() as ctx:
        stat = ctx.enter_context(
            nc.sbuf_tensor("stat_s", [6, 128 * n_pieces], bf16))
        mov = ctx.enter_context(
            nc.sbuf_tensor("mov_s", [6, WP * n_pieces], bf16))
        tmp = ctx.enter_context(nc.sbuf_tensor("tmp_s", [128, n_pieces], f32))
        tmpa = ctx.enter_context(
            nc.sbuf_tensor("tmpa_s", [128, n_pieces], f16))
        act_ring = [ctx.enter_context(
            nc.sbuf_tensor(f"actr{i}", [128, REG["A"][1] // WP, WP], f16))
            for i in range(2)]
        gps_ring = [ctx.enter_context(
            nc.sbuf_tensor(f"gpsr{i}", [128, REG["G"][1] // WP, WP], f16))
            for i in range(2)]
        psum = ctx.enter_context(
            nc.psum_tensor("ps", [128, 4096 // WP, WP], f32))

        stat_sem = ctx.enter_context(nc.semaphore("stat_in"))
        mov_sem = ctx.enter_context(nc.semaphore("mov_in"))
        pe_sem = ctx.enter_context(nc.semaphore("pe_done"))
        dve_sem = ctx.enter_context(nc.semaphore("dve_done"))
        act_sem = ctx.enter_context(nc.semaphore("act_done"))
        gps_sem = ctx.enter_context(nc.semaphore("gps_done"))
        ta_sem = ctx.enter_context(nc.semaphore("tailA"))
        tg_sem = ctx.enter_context(nc.semaphore("tailG"))
        out_sem = ctx.enter_context(nc.semaphore("dma_out"))
        block = ctx.enter_context(nc.Block())

        path_sem = {"D": dve_sem, "A": act_sem, "G": gps_sem}
        # PSUM slot (in WP units) for each piece
        slot_of = {}
        pg_idx = {"D": 0, "A": 0, "G": 0}
        g_meta = []  # (path, lo, hi, slot0, path_group_idx)
        for path, lo, hi in groups:
            k = pg_idx[path]
            base = (REG[path][0] + (k % 2) * REG[path][1]) // WP
            for i in range(lo, hi):
                slot_of[i] = base + (i - lo)
            g_meta.append((path, lo, hi, base, k))
            pg_idx[path] += 1

        chunk_of_piece = np.zeros(n_pieces, np.int64)
        for c, (p0, p1) in enumerate(mov_chunks):
            chunk_of_piece[p0:p1] = c

        @block.sync
        def _(sync):
            sync.dma_start(stat[:], stat_d[:]).then_inc(stat_sem, 16)
            for (p0, p1) in mov_chunks:
                sync.dma_start(mov[:, p0 * WP:p1 * WP],
                               mov_d[:, p0 * WP:p1 * WP]).then_inc(mov_sem, 16)
            sync.wait_ge(dve_sem, n_d)
            sync.wait_ge(ta_sem, n_a)
            sync.wait_ge(tg_sem, n_g)
            sync.dma_start(tmp_d[:], tmp[:]).then_inc(out_sem, 16)
            sync.dma_start(tmpa_d[:], tmpa[:]).then_inc(out_sem, 16)

        @block.tensor
        def _(tensor):
            tensor.wait_ge(stat_sem, 16)
            seen_chunk = -1
            for path, lo, hi, base, k in g_meta:
                if k >= 2:
                    tensor.wait_ge(path_sem[path], k - 1)
                for i in range(lo, hi):
                    c = int(chunk_of_piece[i])
                    if c > seen_chunk:
                        tensor.wait_ge(mov_sem, 16 * (c + 1))
                        seen_chunk = c
                    nc.tensor.matmul(
                        psum[:, slot_of[i], :],
                        stat[:, i * 128:(i + 1) * 128],
                        mov[:, i * WP:(i + 1) * WP],
                        start=True, stop=True,
                    ).then_inc(pe_sem, 1)

        if n_a:
            @block.scalar
            def _(scalar):
                for path, lo, hi, base, k in g_meta:
                    if path != "A":
                        continue
                    scalar.wait_ge(pe_sem, hi)
                    if k >= 2:
                        scalar.wait_ge(ta_sem, k - 1)
                    t = hi - lo
                    src = psum[:, base:base + t, :]
                    dst = act_ring[k % 2][:, 0:t, :]
                    nc.scalar.activation(
                        dst.rearrange("p a b -> p (a b)"),
                        src.rearrange("p a b -> p (a b)"),
                        mybir.ActivationFunctionType.Copy, scale=D2_SCALE,
                    ).then_inc(act_sem, 1)

        if n_g:
            @block.gpsimd
            def _(gpsimd):
                MIN = mybir.AluOpType.min
                MUL = mybir.AluOpType.mult
                for path, lo, hi, base, k in g_meta:
                    if path != "G":
                        continue
                    gpsimd.wait_ge(pe_sem, hi)
                    if k >= 2:
                        gpsimd.wait_ge(tg_sem, k - 1)
                    t = hi - lo
                    src = psum[:, base:base + t, :]
                    dst = gps_ring[k % 2][:, 0:t, :]
                    nc.gpsimd.tensor_scalar(
                        dst.rearrange("p a b -> p (a b)"),
                        src.rearrange("p a b -> p (a b)"),
                        0.0, D2_SCALE, MIN, MUL,
                    ).then_inc(gps_sem, 1)

        @block.vector
        def _(vector):
            for path, lo, hi, base, k in g_meta:
                t = hi - lo
                if path == "D":
                    vector.wait_ge(pe_sem, hi)
                    nc.vector.tensor_reduce(
                        tmp[:, lo:hi], psum[:, base:base + t, :],
                        axis=X, op=MAX,
                    ).then_inc(dve_sem, 1)
                elif path == "A":
                    vector.wait_ge(act_sem, k + 1)
                    nc.vector.tensor_reduce(
                        tmpa[:, lo:hi], act_ring[k % 2][:, 0:t, :],
                        axis=X, op=MAX,
                    ).then_inc(ta_sem, 1)
                elif path == "G":
                    vector.wait_ge(gps_sem, k + 1)
                    nc.vector.tensor_reduce(
                        tmpa[:, lo:hi], gps_ring[k % 2][:, 0:t, :],
                        axis=X, op=MAX,
                    ).then_inc(tg_sem, 1)

    return nc, groups


def _loss_from_sums(sg, ng, sp, npnts):
    with np.errstate(divide="ignore", invalid="ignore"):
        g2p = sg / ng if ng > 0 else np.float64(np.nan)
        p2g = sp / npnts if npnts > 0 else np.float64(np.nan)
        if ng == 0 and npnts == 0:
            return np.float64(np.nan)
        ahd = (g2p + p2g) / 2.0
        return 1.0 - 1.0 / (1.0 + ahd)


RUN_OPTS = {}
LAST_RES = None
LAST_NN = None


def kernel(gth, pred):
    from concourse.bass_utils import run_bass_kernel_spmd
    import ml_dtypes

    gth = np.asarray(gth, np.float32).reshape(BC, H, W_IMG)
    pred = np.asarray(pred, np.float32).reshape(BC, H, W_IMG)

    gedge = _edge_maps(gth)
    pedge = _edge_maps(pred)

    # all jobs: (n_pieces, pair, dir, q_idx, cand_idx)
    all_jobs = []
    pts = []
    for i in range(BC):
        gy, gx = np.nonzero(gedge[i])
        py, px = np.nonzero(pedge[i])
        pts.append((gy.astype(np.float32) - 128.0, gx.astype(np.float32) - 128.0,
                    py.astype(np.float32) - 128.0, px.astype(np.float32) - 128.0))
        if len(gy) and len(py):
            for d, (qys, qxs, dys, dxs) in enumerate(
                    [(gy, gx, py, px), (py, px, gy, gx)]):
                for q_ix, c_ix in _build_jobs(qys, qxs, dys, dxs):
                    npc = max(1, -(-len(c_ix) // WP))
                    all_jobs.append((npc, i, d, q_ix, c_ix))

    # bin-pack jobs across cores by piece count
    order = sorted(range(len(all_jobs)),
                   key=lambda k: all_jobs[k][0], reverse=True)
    loads = [0] * N_CORES
    per_core = [[] for _ in range(N_CORES)]
    for k in order:
        c = min(range(N_CORES), key=lambda q: loads[q])
        per_core[c].append(k)
        loads[c] += all_jobs[k][0]
    P = max(loads)

    # moving-data DMA chunks (piece ranges), ~4 equal spans
    bounds = sorted(set([0] + [P * t // 4 for t in range(1, 4)] + [P]))
    mov_chunks = [(bounds[t], bounds[t + 1]) for t in range(len(bounds) - 1)]

    nc, groups = _build_program(P, mov_chunks)
    path_of_piece = {}
    for path, lo, hi in groups:
        for i in range(lo, hi):
            path_of_piece[i] = path

    # per-core inputs; piece layout: jobs in per_core order, consecutive
    sent_stat = _aug_g(np.empty(0, np.float32), np.empty(0, np.float32), 128)
    sent_mov = _aug_p(np.empty(0, np.float32), np.empty(0, np.float32), WP)
    in_maps = []
    piece_map = []  # per core: list of (job_key or None) per piece
    for c in range(N_CORES):
        stat = np.empty((6, 128 * P), np.float32)
        mov = np.empty((6, WP * P), np.float32)
        pmap = []
        p = 0
        for k in per_core[c]:
            npc, i, d, q_ix, c_ix = all_jobs[k]
            gy, gx, py, px = pts[i]
            if d == 0:
                qys, qxs, dys, dxs = gy, gx, py, px
            else:
                qys, qxs, dys, dxs = py, px, gy, gx
            sa = _aug_g(qys[q_ix], qxs[q_ix], 128)
            aug = _aug_p(dys[c_ix], dxs[c_ix], npc * WP)
            for t in range(npc):
                stat[:, (p + t) * 128:(p + t + 1) * 128] = sa
                mov[:, (p + t) * WP:(p + t + 1) * WP] = \
                    aug[:, t * WP:(t + 1) * WP]
                pmap.append(k)
            p += npc
        while p < P:
            stat[:, p * 128:(p + 1) * 128] = sent_stat
            mov[:, p * WP:(p + 1) * WP] = sent_mov
            pmap.append(None)
            p += 1
        piece_map.append(pmap)
        in_maps.append({
            "stat": stat.astype(ml_dtypes.bfloat16),
            "mov": mov.astype(ml_dtypes.bfloat16),
        })

    res = run_bass_kernel_spmd(nc, in_maps, list(range(N_CORES)), **RUN_OPTS)
    global LAST_RES, LAST_NN
    LAST_RES = res

    # decode: per piece column -> min d^2; per job: min over pieces
    sums = np.zeros((BC, 2), np.float64)
    nn_dbg = {}
    for c in range(N_CORES):
        tmpv = np.asarray(res.results[c]["tmp"], np.float64)    # [128, P]
        tmpav = np.asarray(res.results[c]["tmpa"], np.float64)  # [128, P]
        d2col = np.empty((128, P), np.float64)
        for i in range(P):
            if path_of_piece[i] in ("A", "G"):
                d2col[:, i] = tmpav[:, i] * (-4.0 * 4096.0)
            else:
                d2col[:, i] = tmpv[:, i] * -4.0
        # gather pieces per job
        job_d2 = {}
        for i, k in enumerate(piece_map[c]):
            if k is None:
                continue
            cur = job_d2.get(k)
            job_d2[k] = d2col[:, i] if cur is None \
                else np.minimum(cur, d2col[:, i])
        for k, d2 in job_d2.items():
            npc, i, d, q_ix, c_ix = all_jobs[k]
            dist = np.sqrt(np.maximum(d2[:len(q_ix)], 0.0))
            sums[i, d] += dist.sum()
            nn_dbg.setdefault((i, d), []).append((q_ix, dist))
    LAST_NN = nn_dbg

    losses = np.full(BC, np.nan, np.float64)
    for i in range(BC):
        gy = pts[i][0]
        py = pts[i][2]
        n_g, n_p = len(gy), len(py)
        if n_g == 0 and n_p == 0:
            continue
        if n_g == 0 or n_p == 0:
            losses[i] = _loss_from_sums(np.inf, max(n_g, 1),
                                        np.inf, max(n_p, 1))
        else:
            losses[i] = _loss_from_sums(sums[i, 0], n_g, sums[i, 1], n_p)

    return np.float32(np.nanmean(losses.astype(np.float32)))


# revision 19
# speedup vs baseline: 3.1591x; 1.0163x over previous
"""Average Hausdorff loss on 8 Trainium2 NeuronCores.

Strategy (v3: pruned two-direction NN, measured-rate engine balance)
-------------------------------------------------------------------
Host (numpy, cheap):
  * binarize + 3x3-erosion edge detection, compact edge coords per (b,c)
  * per direction (g->p and p->g): KD-split the query points into tiles
    of <=128; a cell-grid separable EDT gives a per-point upper bound on
    the NN distance; each tile's candidate set = DB points within the
    per-16-query-subblock bbox expanded by the subblock's max bound.
    The candidate set provably contains every query's true NN.
  * each (tile, candidates) job is cut into uniform 128-candidate
    PIECES. Pieces are bin-packed across the 8 cores; the device
    program is piece-index uniform (all per-core variation lives in the
    DMAed data: each piece has its own stationary copy + moving block).

Device (raw Bass, SPMD over 8 cores):
  PE   : per job, one matmul [6,128]^T @ [6,W] -> PSUM -(d^2)/4 (exact
         in bf16 via byte-split squared norms), split at group bounds.
         A junk-input warmup stream runs during the DMA head so the PE
         HAM clock-gate reaches 2.4 GHz before real work.
  Pieces are consumed in GROUPS of 8 (1024 cols) by two statically
  assigned reduce paths (PSUM split into 2 double-buffered regions):
    D: DVE tensor_reduce [128,8,128] -> [128,8] f32 straight from PSUM
    A: ACT copies the group to SBUF fp16 (scale 2^-12); DVE does a
       fp16 tensor_max fold (2x mode) + [128,8,64] tensor_reduce
  Each piece yields one temp column (per-query max of -(d^2)/4).
Host: per job take max over its pieces' columns -> NN distance per
query point; sqrt, masked means, nanmean.
"""

import math
import numpy as np

H = 256
W_IMG = 256
BC = 16
N_CORES = 8
TILE_Q = 128          # query points per job
SUB_Q = 16            # sub-block size for candidate bbox union
CELL = 2              # bound-grid cell size in px
WP = 128              # uniform piece width (candidate cols)
GT = 8                # pieces per reduce group (1024 cols)
SENT = 16384.0        # sentinel coordinate (centered space)
D2_SCALE = 2.0 ** -12
PATHS = "DA"
N_WARM = 14           # junk warmup matmuls (256 cols each)

# measured per-group costs (ns) at W = GT*WP = 1024 cols
COST_D_DVE = (1210.0, 150.0)       # slope/col, fixed
COST_A_ACT = (1160.0, 160.0)
COST_A_TAIL = (1097.0, 240.0)      # fold + reduce on DVE


def _edge_maps(x):
    m = x > 0.5
    p = np.pad(m, ((0, 0), (1, 1), (1, 1)), constant_values=True)
    e = np.ones_like(m)
    for dy in range(3):
        for dx in range(3):
            e &= p[:, dy:dy + H, dx:dx + W_IMG]
    return m & ~e


def _aug_g(cy, cx, n_pad):
    n = cy.shape[0]
    fy = np.full(n_pad, SENT, np.float32)
    fx = np.full(n_pad, SENT, np.float32)
    fy[:n] = cy
    fx[:n] = cx
    sq = fy * fy + fx * fx
    b1 = np.floor(sq / 256.0)
    b0 = sq - b1 * 256.0
    out = np.empty((6, n_pad), np.float32)
    out[0] = fy * 0.5
    out[1] = fx * 0.5
    out[2] = -b1
    out[3] = -b0
    out[4] = -64.0
    out[5] = -0.25
    return out


def _aug_p(cy, cx, n_pad):
    n = cy.shape[0]
    fy = np.full(n_pad, SENT, np.float32)
    fx = np.full(n_pad, SENT, np.float32)
    fy[:n] = cy
    fx[:n] = cx
    sq = fy * fy + fx * fx
    b1 = np.floor(sq / 256.0)
    b0 = sq - b1 * 256.0
    out = np.empty((6, n_pad), np.float32)
    out[0] = fy
    out[1] = fx
    out[2] = 64.0
    out[3] = 0.25
    out[4] = b1
    out[5] = b0
    return out


def _kd_tiles(ys, xs, tile):
    out = []

    def rec(ix):
        if len(ix) <= tile:
            out.append(ix)
            return
        yy, xx = ys[ix], xs[ix]
        k = yy if (yy.max() - yy.min() >= xx.max() - xx.min()) else xx
        n = len(ix)
        half = (n // 2 // tile) * tile or n // 2
        o = np.argsort(k, kind="stable")
        rec(ix[o[:half]])
        rec(ix[o[half:]])

    rec(np.arange(len(ys)))
    return out


def _cell_ub(dys, dxs):
    G = 256 // CELL
    occ = np.zeros((G, G), bool)
    occ[dys // CELL, dxs // CELL] = True
    BIG = np.int64(10 ** 9)
    ar = np.arange(G)
    d2 = (ar[:, None] - ar[None, :]) ** 2
    occf = np.where(occ, 0, BIG)
    gcol = (d2[:, :, None] + occf[None, :, :]).min(axis=1)
    D2 = (gcol[:, None, :] + d2[None, :, :]).min(axis=2)
    return np.sqrt(D2.astype(np.float64)) * CELL + math.sqrt(2.0) * CELL


def _build_jobs(qys, qxs, dys, dxs):
    ub = _cell_ub(dys, dxs)[qys // CELL, qxs // CELL]
    jobs = []
    for ix in _kd_tiles(qys, qxs, TILE_Q):
        m = np.zeros(len(dys), bool)
        for s in range(0, len(ix), SUB_Q):
            sx = ix[s:s + SUB_Q]
            u = ub[sx].max()
            y0, y1 = qys[sx].min() - u, qys[sx].max() + u
            x0, x1 = qxs[sx].min() - u, qxs[sx].max() + u
            m |= (dys >= y0) & (dys <= y1) & (dxs >= x0) & (dxs <= x1)
        jobs.append((ix, np.nonzero(m)[0]))
    return jobs


def _plan_groups(n_pieces):
    """Assign consecutive GT-piece groups to paths, balancing measured
    engine busy-ns. Returns list of (path, piece_lo, piece_hi)."""
    busy = {"DVE": 0.0, "ACT": 0.0}
    groups = []
    p = 0
    while p < n_pieces:
        t = min(GT, n_pieces - p)
        w = t * WP
        cand = {}
        if "D" in PATHS:
            c = w / 1024 * COST_D_DVE[0] + COST_D_DVE[1]
            cand["D"] = (busy["DVE"] + c, (("DVE", c),))
        if "A" in PATHS:
            ca = w / 1024 * COST_A_ACT[0] + COST_A_ACT[1]
            cd = w / 1024 * COST_A_TAIL[0] + COST_A_TAIL[1]
            m = max(busy["ACT"] + ca, busy["DVE"] + cd)
            cand["A"] = (m, (("ACT", ca), ("DVE", cd)))
        best = min(cand, key=lambda k: cand[k][0])
        for eng, c in cand[best][1]:
            busy[eng] += c
        groups.append((best, p, p + t))
        p += t
    return groups


def _build_program(n_pieces, piece_job, mov_chunks_sync, mov_chunks_gps):
    """Raw-bass SPMD program, uniform over piece index.

    piece_job: job id per piece (pieces of one job are consecutive);
    matmuls are merged across consecutive pieces of the same job within
    a group. mov_chunks_*: piece ranges DMAed by sync / gpsimd engines.
    """
    from contextlib import ExitStack
    import concourse.bass as bass
    import concourse.mybir as mybir

    f32 = mybir.dt.float32
    f16 = mybir.dt.float16
    bf16 = mybir.dt.bfloat16
    MAX = mybir.AluOpType.max
    X = mybir.AxisListType.X

    groups = _plan_groups(n_pieces)

    nc = bass.Bass()
    stat_d = nc.declare_dram_parameter("stat", [6, 128 * n_pieces], bf16,
                                       isOutput=False)
    mov_d = nc.declare_dram_parameter("mov", [6, WP * n_pieces], bf16,
                                      isOutput=False)
    tmp_d = nc.declare_dram_parameter("tmp", [128, n_pieces], f32,
                                      isOutput=True)
    tmpa_d = nc.declare_dram_parameter("tmpa", [128, n_pieces], f16,
                                       isOutput=True)

    n_a = sum(1 for g in groups if g[0] == "A")
    n_d = sum(1 for g in groups if g[0] == "D")

    # merged matmul list per group: runs of same-job pieces, split at
    # PSUM bank boundaries (4 pieces = 512 f32) relative to group base
    def group_matmuls(lo, hi):
        mms = []
        i = lo
        while i < hi:
            j = i
            while (j + 1 < hi and piece_job[j + 1] == piece_job[i]
                   and (j + 1 - lo) % 4 != 0):
                j += 1
            mms.append((i, j - i + 1))
            i = j + 1
        return mms

    with ExitStack() as ctx:
        stat = ctx.enter_context(
            nc.sbuf_tensor("stat_s", [6, 128 * n_pieces], bf16))
        mov = ctx.enter_context(
            nc.sbuf_tensor("mov_s", [6, WP * n_pieces], bf16))
        tmp = ctx.enter_context(nc.sbuf_tensor("tmp_s", [128, n_pieces], f32))
        tmpa = ctx.enter_context(
            nc.sbuf_tensor("tmpa_s", [128, n_pieces], f16))
        act_ring = [ctx.enter_context(
            nc.sbuf_tensor(f"actr{i}", [128, GT, WP], f16)) for i in range(2)]
        fold = ctx.enter_context(
            nc.sbuf_tensor("fold_s", [128, GT, WP // 2], f16))
        psum = ctx.enter_context(
            nc.psum_tensor("ps", [128, 4096 // WP, WP], f32))

        stat_sem = ctx.enter_context(nc.semaphore("stat_in"))
        mov_sem = ctx.enter_context(nc.semaphore("mov_in"))
        mov2_sem = ctx.enter_context(nc.semaphore("mov2_in"))
        pe_sem = ctx.enter_context(nc.semaphore("pe_done"))
        dve_sem = ctx.enter_context(nc.semaphore("dve_done"))
        act_sem = ctx.enter_context(nc.semaphore("act_done"))
        ta_sem = ctx.enter_context(nc.semaphore("tailA"))
        out_sem = ctx.enter_context(nc.semaphore("dma_out"))
        block = ctx.enter_context(nc.Block(no_gpsimd_drain=True))

        path_sem = {"D": dve_sem, "A": act_sem}
        # PSUM regions: D slots 0..15 in units of WP; D ping-pong at
        # units [0,8) / [8,16); A at [16,24) / [24,32)
        pg_idx = {"D": 0, "A": 0}
        g_meta = []  # (path, lo, hi, slot0, path_group_idx, cum_mm)
        cum = 0
        for path, lo, hi in groups:
            k = pg_idx[path]
            base = (0 if path == "D" else 2 * GT) + (k % 2) * GT
            cum += 1
            g_meta.append((path, lo, hi, base, k, cum))
            pg_idx[path] += 1

        # DMA chunk thresholds per piece for the two loader engines
        sync_cnt = np.zeros(n_pieces, np.int64)
        gps_cnt = np.zeros(n_pieces, np.int64)
        for (p0, p1) in mov_chunks_sync:
            sync_cnt[p0:] += 1
        for (p0, p1) in mov_chunks_gps:
            gps_cnt[p0:] += 1
        # a piece needs all chunks covering pieces <= itself: since
        # chunks are contiguous from 0, count chunks whose end > piece
        sync_need = np.zeros(n_pieces, np.int64)
        gps_need = np.zeros(n_pieces, np.int64)
        for c, (p0, p1) in enumerate(mov_chunks_sync):
            sync_need[p0:p1] = c + 1
        for c, (p0, p1) in enumerate(mov_chunks_gps):
            gps_need[p0:p1] = c + 1
        sync_need = np.maximum.accumulate(sync_need)
        gps_need = np.maximum.accumulate(gps_need)

        @block.sync
        def _(sync):
            sync.dma_start(stat[:], stat_d[:]).then_inc(stat_sem, 16)
            for (p0, p1) in mov_chunks_sync:
                sync.dma_start(mov[:, p0 * WP:p1 * WP],
                               mov_d[:, p0 * WP:p1 * WP]).then_inc(mov_sem, 16)
            sync.wait_ge(dve_sem, n_d)
            sync.wait_ge(ta_sem, n_a)
            sync.dma_start(tmp_d[:], tmp[:]).then_inc(out_sem, 16)
            sync.dma_start(tmpa_d[:], tmpa[:]).then_inc(out_sem, 16)

        @block.gpsimd
        def _(gpsimd):
            for (p0, p1) in mov_chunks_gps:
                gpsimd.dma_start(
                    mov[:, p0 * WP:p1 * WP],
                    mov_d[:, p0 * WP:p1 * WP]).then_inc(mov2_sem, 16)

        @block.tensor
        def _(tensor):
            # warmup stream on junk SBUF data -> PSUM region A slot 0;
            # no deps, runs during preamble+DMA head to heat the HAM
            for wmm in range(N_WARM):
                nc.tensor.matmul(
                    psum[:, 2 * GT + (wmm % 2), :],
                    stat[:, 0:128], mov[:, 0:WP],
                    start=True, stop=True,
                )
            tensor.wait_ge(stat_sem, 16)
            s_seen = 0
            g_seen = 0
            hw_mark = {"D": 0, "A": 0}
            for path, lo, hi, base, k, cum in g_meta:
                if k >= 2 and k - 1 > hw_mark[path]:
                    tensor.wait_ge(path_sem[path], k - 1)
                    hw_mark[path] = k - 1
                need_s = int(sync_need[hi - 1])
                need_g = int(gps_need[hi - 1])
                if need_s > s_seen:
                    tensor.wait_ge(mov_sem, 16 * need_s)
                    s_seen = need_s
                if need_g > g_seen:
                    tensor.wait_ge(mov2_sem, 16 * need_g)
                    g_seen = need_g
                mms = group_matmuls(lo, hi)
                for mi, (plo, np_) in enumerate(mms):
                    slot = base + (plo - lo)
                    mm = nc.tensor.matmul(
                        psum[:].rearrange("p a b -> p (a b)")
                            [:, slot * WP:(slot + np_) * WP],
                        stat[:, plo * 128:(plo + 1) * 128],
                        mov[:, plo * WP:(plo + np_) * WP],
                        start=True, stop=True,
                    )
                    if mi == len(mms) - 1:
                        mm.then_inc(pe_sem, 1)

        if n_a:
            @block.scalar
            def _(scalar):
                for path, lo, hi, base, k, cum in g_meta:
                    if path != "A":
                        continue
                    scalar.wait_ge(pe_sem, cum)
                    if k >= 2:
                        scalar.wait_ge(ta_sem, k - 1)
                    t = hi - lo
                    src = psum[:, base:base + t, :]
                    dst = act_ring[k % 2][:, 0:t, :]
                    nc.scalar.activation(
                        dst.rearrange("p a b -> p (a b)"),
                        src.rearrange("p a b -> p (a b)"),
                        mybir.ActivationFunctionType.Copy, scale=D2_SCALE,
                    ).then_inc(act_sem, 1)

        @block.vector
        def _(vector):
            for path, lo, hi, base, k, cum in g_meta:
                t = hi - lo
                if path == "D":
                    vector.wait_ge(pe_sem, cum)
                    nc.vector.tensor_reduce(
                        tmp[:, lo:hi], psum[:, base:base + t, :],
                        axis=X, op=MAX,
                    ).then_inc(dve_sem, 1)
                else:
                    vector.wait_ge(act_sem, k + 1)
                    buf = act_ring[k % 2]
                    nc.vector.tensor_max(
                        fold[:, 0:t, :],
                        buf[:, 0:t, 0:WP // 2], buf[:, 0:t, WP // 2:WP],
                    )
                    nc.vector.tensor_reduce(
                        tmpa[:, lo:hi], fold[:, 0:t, :],
                        axis=X, op=MAX,
                    ).then_inc(ta_sem, 1)

    return nc, groups


def _loss_from_sums(sg, ng, sp, npnts):
    with np.errstate(divide="ignore", invalid="ignore"):
        g2p = sg / ng if ng > 0 else np.float64(np.nan)
        p2g = sp / npnts if npnts > 0 else np.float64(np.nan)
        if ng == 0 and npnts == 0:
            return np.float64(np.nan)
        ahd = (g2p + p2g) / 2.0
        return 1.0 - 1.0 / (1.0 + ahd)


RUN_OPTS = {}
LAST_RES = None
LAST_NN = None


def kernel(gth, pred):
    from concourse.bass_utils import run_bass_kernel_spmd
    import ml_dtypes

    gth = np.asarray(gth, np.float32).reshape(BC, H, W_IMG)
    pred = np.asarray(pred, np.float32).reshape(BC, H, W_IMG)

    gedge = _edge_maps(gth)
    pedge = _edge_maps(pred)

    all_jobs = []
    pts = []
    for i in range(BC):
        gy, gx = np.nonzero(gedge[i])
        py, px = np.nonzero(pedge[i])
        pts.append((gy.astype(np.float32) - 128.0, gx.astype(np.float32) - 128.0,
                    py.astype(np.float32) - 128.0, px.astype(np.float32) - 128.0))
        if len(gy) and len(py):
            for d, (qys, qxs, dys, dxs) in enumerate(
                    [(gy, gx, py, px), (py, px, gy, gx)]):
                for q_ix, c_ix in _build_jobs(qys, qxs, dys, dxs):
                    npc = max(1, -(-len(c_ix) // WP))
                    all_jobs.append((npc, i, d, q_ix, c_ix))

    order = sorted(range(len(all_jobs)),
                   key=lambda k: all_jobs[k][0], reverse=True)
    loads = [0] * N_CORES
    per_core = [[] for _ in range(N_CORES)]
    for k in order:
        c = min(range(N_CORES), key=lambda q: loads[q])
        per_core[c].append(k)
        loads[c] += all_jobs[k][0]

    # uniform job slots: sort each core's jobs desc, slot width = max
    # pieces over cores at that position -> piece/job layout identical
    # on every core (smaller jobs padded with sentinel pieces)
    for c in range(N_CORES):
        per_core[c].sort(key=lambda k: all_jobs[k][0], reverse=True)
    J = max(len(v) for v in per_core)
    slot_w = [0] * J
    for c in range(N_CORES):
        for j, k in enumerate(per_core[c]):
            slot_w[j] = max(slot_w[j], all_jobs[k][0])
    P = sum(slot_w)
    piece_job = np.zeros(P, np.int64)
    slot_off = []
    p = 0
    for j, w in enumerate(slot_w):
        slot_off.append(p)
        piece_job[p:p + w] = j
        p += w

    # DMA chunks: sync engine loads pieces [0, P//2), gpsimd the rest,
    # each split in two
    q1, q2, q3 = P // 4, P // 2, (3 * P) // 4
    mov_chunks_sync = [(0, q1), (q1, q2)]
    mov_chunks_gps = [(q2, q3), (q3, P)]

    nc, groups = _build_program(P, piece_job, mov_chunks_sync, mov_chunks_gps)
    path_of_piece = {}
    for path, lo, hi in groups:
        for i in range(lo, hi):
            path_of_piece[i] = path

    sent_stat = _aug_g(np.empty(0, np.float32), np.empty(0, np.float32), 128)
    sent_mov = _aug_p(np.empty(0, np.float32), np.empty(0, np.float32), WP)
    in_maps = []
    piece_map = []
    for c in range(N_CORES):
        stat = np.empty((6, 128 * P), np.float32)
        mov = np.empty((6, WP * P), np.float32)
        pmap = [None] * P
        for j in range(J):
            p = slot_off[j]
            w = slot_w[j]
            if j < len(per_core[c]):
                k = per_core[c][j]
                npc, i, d, q_ix, c_ix = all_jobs[k]
                gy, gx, py, px = pts[i]
                if d == 0:
                    qys, qxs, dys, dxs = gy, gx, py, px
                else:
                    qys, qxs, dys, dxs = py, px, gy, gx
                sa = _aug_g(qys[q_ix], qxs[q_ix], 128)
                aug = _aug_p(dys[c_ix], dxs[c_ix], w * WP)
                for t in range(w):
                    stat[:, (p + t) * 128:(p + t + 1) * 128] = sa
                    mov[:, (p + t) * WP:(p + t + 1) * WP] = \
                        aug[:, t * WP:(t + 1) * WP]
                    pmap[p + t] = k
            else:
                for t in range(w):
                    stat[:, (p + t) * 128:(p + t + 1) * 128] = sent_stat
                    mov[:, (p + t) * WP:(p + t + 1) * WP] = sent_mov
        piece_map.append(pmap)
        in_maps.append({
            "stat": stat.astype(ml_dtypes.bfloat16),
            "mov": mov.astype(ml_dtypes.bfloat16),
        })

    res = run_bass_kernel_spmd(nc, in_maps, list(range(N_CORES)), **RUN_OPTS)
    global LAST_RES, LAST_NN
    LAST_RES = res

    sums = np.zeros((BC, 2), np.float64)
    nn_dbg = {}
    for c in range(N_CORES):
        tmpv = np.asarray(res.results[c]["tmp"], np.float64)
        tmpav = np.asarray(res.results[c]["tmpa"], np.float64)
        d2col = np.empty((128, P), np.float64)
        for i in range(P):
            if path_of_piece[i] == "A":
                d2col[:, i] = tmpav[:, i] * (-4.0 * 4096.0)
            else:
                d2col[:, i] = tmpv[:, i] * -4.0
        job_d2 = {}
        for i, k in enumerate(piece_map[c]):
            if k is None:
                continue
            cur = job_d2.get(k)
            job_d2[k] = d2col[:, i] if cur is None \
                else np.minimum(cur, d2col[:, i])
        for k, d2 in job_d2.items():
            npc, i, d, q_ix, c_ix = all_jobs[k]
            dist = np.sqrt(np.maximum(d2[:len(q_ix)], 0.0))
            sums[i, d] += dist.sum()
            nn_dbg.setdefault((i, d), []).append((q_ix, dist))
    LAST_NN = nn_dbg

    losses = np.full(BC, np.nan, np.float64)
    for i in range(BC):
        gy = pts[i][0]
        py = pts[i][2]
        n_g, n_p = len(gy), len(py)
        if n_g == 0 and n_p == 0:
            continue
        if n_g == 0 or n_p == 0:
            losses[i] = _loss_from_sums(np.inf, max(n_g, 1),
                                        np.inf, max(n_p, 1))
        else:
            losses[i] = _loss_from_sums(sums[i, 0], n_g, sums[i, 1], n_p)

    return np.float32(np.nanmean(losses.astype(np.float32)))


# revision 20
# speedup vs baseline: 3.5972x; 1.1387x over previous
"""Average Hausdorff loss on 8 Trainium2 NeuronCores.

Strategy (v4: one-matrix shared blocks, pruned, host dp-finish)
--------------------------------------------------------------
Host (numpy, cheap):
  * binarize + 3x3-erosion edge detection, compact edge coords per (b,c)
  * per pair: KD-split the GTH points into tiles of <=128. A cell-grid
    separable EDT gives per-point NN-distance upper bounds in both
    directions. A tile's candidate set = pred points p with
    dist(p, sub-bbox) <= max(UB_sub, UB_p), which provably contains
      - the NN pred point of every gth query in the tile, and
      - every pred point whose own NN gth point lies in the tile.
    So ONE distance block per gth tile serves both directions.
  * blocks are cut into uniform 128-candidate pieces, bin-packed over
    the 8 cores into uniform job slots (device program is piece-index
    uniform; per-core variation is data only).

Device (raw Bass, SPMD over 8 cores):
  PE  : merged matmuls [6,128]^T @ [6,<=512] -> PSUM -(d^2)/4 (exact
        bf16 via byte-split squared norms), 4 ping-pong PSUM regions.
  ACT : copies each 8-piece group to an SBUF fp16 ring (scale 2^-12).
  DVE : fp16 tensor_max fold + [128,8,64] tensor_reduce -> per-gth-
        query NN column per piece (g->p direction).
  DMA : streams every drained fp16 block back to DRAM.
Host: g->p from the NN columns; p->g by per-column max over the 128
partitions of the returned blocks, scatter-min by candidate id; sqrt,
means, nanmean.
"""

import math
import numpy as np

H = 256
W_IMG = 256
BC = 16
N_CORES = 8
TILE_Q = 128
SUB_Q = 16
CELL = 2
WP = 128              # uniform piece width (candidate cols)
GT = 8                # pieces per reduce group (1024 cols)
SENT = 16384.0
D2_SCALE = 2.0 ** -12
D2_BACK = -4.0 * 4096.0


def _edge_maps(x):
    m = x > 0.5
    p = np.pad(m, ((0, 0), (1, 1), (1, 1)), constant_values=True)
    e = np.ones_like(m)
    for dy in range(3):
        for dx in range(3):
            e &= p[:, dy:dy + H, dx:dx + W_IMG]
    return m & ~e


def _aug_g(cy, cx, n_pad):
    n = cy.shape[0]
    fy = np.full(n_pad, SENT, np.float32)
    fx = np.full(n_pad, SENT, np.float32)
    fy[:n] = cy
    fx[:n] = cx
    sq = fy * fy + fx * fx
    b1 = np.floor(sq / 256.0)
    b0 = sq - b1 * 256.0
    out = np.empty((6, n_pad), np.float32)
    out[0] = fy * 0.5
    out[1] = fx * 0.5
    out[2] = -b1
    out[3] = -b0
    out[4] = -64.0
    out[5] = -0.25
    return out


def _aug_p(cy, cx, n_pad):
    n = cy.shape[0]
    fy = np.full(n_pad, SENT, np.float32)
    fx = np.full(n_pad, SENT, np.float32)
    fy[:n] = cy
    fx[:n] = cx
    sq = fy * fy + fx * fx
    b1 = np.floor(sq / 256.0)
    b0 = sq - b1 * 256.0
    out = np.empty((6, n_pad), np.float32)
    out[0] = fy
    out[1] = fx
    out[2] = 64.0
    out[3] = 0.25
    out[4] = b1
    out[5] = b0
    return out


def _kd_tiles(ys, xs, tile):
    out = []

    def rec(ix):
        if len(ix) <= tile:
            out.append(ix)
            return
        yy, xx = ys[ix], xs[ix]
        k = yy if (yy.max() - yy.min() >= xx.max() - xx.min()) else xx
        n = len(ix)
        half = (n // 2 // tile) * tile or n // 2
        o = np.argsort(k, kind="stable")
        rec(ix[o[:half]])
        rec(ix[o[half:]])

    rec(np.arange(len(ys)))
    return out


def _cell_ub(dys, dxs):
    G = 256 // CELL
    occ = np.zeros((G, G), bool)
    occ[dys // CELL, dxs // CELL] = True
    BIG = np.int64(10 ** 9)
    ar = np.arange(G)
    d2 = (ar[:, None] - ar[None, :]) ** 2
    occf = np.where(occ, 0, BIG)
    gcol = (d2[:, :, None] + occf[None, :, :]).min(axis=1)
    D2 = (gcol[:, None, :] + d2[None, :, :]).min(axis=2)
    return np.sqrt(D2.astype(np.float64)) * CELL + math.sqrt(2.0) * CELL


def _build_jobs_shared(gy, gx, py, px):
    """Per pair: one job per gth tile; candidates serve both directions."""
    ub_g2p = _cell_ub(py, px)[gy // CELL, gx // CELL]
    ub_p2g = _cell_ub(gy, gx)[py // CELL, px // CELL]
    jobs = []
    for ix in _kd_tiles(gy, gx, TILE_Q):
        m = np.zeros(len(py), bool)
        for s in range(0, len(ix), SUB_Q):
            sx = ix[s:s + SUB_Q]
            u = ub_g2p[sx].max()
            y0, y1 = gy[sx].min(), gy[sx].max()
            x0, x1 = gx[sx].min(), gx[sx].max()
            dy = np.maximum(np.maximum(y0 - py, py - y1), 0)
            dx = np.maximum(np.maximum(x0 - px, px - x1), 0)
            dd = np.sqrt((dy * dy + dx * dx).astype(np.float64))
            m |= (dd <= u) | (dd <= ub_p2g)
        jobs.append((ix, np.nonzero(m)[0]))
    return jobs


def _build_program(n_pieces, piece_job, mov_chunks_sync, mov_chunks_gps):
    from contextlib import ExitStack
    import concourse.bass as bass
    import concourse.mybir as mybir

    f32 = mybir.dt.float32
    f16 = mybir.dt.float16
    bf16 = mybir.dt.bfloat16
    MAX = mybir.AluOpType.max
    X = mybir.AxisListType.X

    groups = []
    p = 0
    while p < n_pieces:
        t = min(GT, n_pieces - p)
        groups.append((p, p + t))
        p += t
    n_grp = len(groups)

    nc = bass.Bass()
    stat_d = nc.declare_dram_parameter("stat", [6, 128 * n_pieces], bf16,
                                       isOutput=False)
    mov_d = nc.declare_dram_parameter("mov", [6, WP * n_pieces], bf16,
                                      isOutput=False)
    tmpa_d = nc.declare_dram_parameter("tmpa", [128, n_pieces], f16,
                                       isOutput=True)
    blk_d = nc.declare_dram_parameter("blk", [128, WP * n_pieces], f16,
                                      isOutput=True)

    def group_matmuls(lo, hi):
        mms = []
        i = lo
        while i < hi:
            j = i
            while (j + 1 < hi and piece_job[j + 1] == piece_job[i]
                   and (j + 1 - lo) % 4 != 0):
                j += 1
            mms.append((i, j - i + 1))
            i = j + 1
        return mms

    with ExitStack() as ctx:
        stat = ctx.enter_context(
            nc.sbuf_tensor("stat_s", [6, 128 * n_pieces], bf16))
        mov = ctx.enter_context(
            nc.sbuf_tensor("mov_s", [6, WP * n_pieces], bf16))
        tmpa = ctx.enter_context(
            nc.sbuf_tensor("tmpa_s", [128, n_pieces], f16))
        act_ring = [ctx.enter_context(
            nc.sbuf_tensor(f"actr{i}", [128, GT, WP], f16)) for i in range(2)]
        fold = ctx.enter_context(
            nc.sbuf_tensor("fold_s", [128, GT, WP // 2], f16))
        psum = ctx.enter_context(
            nc.psum_tensor("ps", [128, 4096 // WP, WP], f32))

        stat_sem = ctx.enter_context(nc.semaphore("stat_in"))
        mov_sem = ctx.enter_context(nc.semaphore("mov_in"))
        mov2_sem = ctx.enter_context(nc.semaphore("mov2_in"))
        pe_sem = ctx.enter_context(nc.semaphore("pe_done"))
        act_sem = ctx.enter_context(nc.semaphore("act_done"))
        ta_sem = ctx.enter_context(nc.semaphore("tailA"))
        blk_sem = ctx.enter_context(nc.semaphore("blk_out"))
        out_sem = ctx.enter_context(nc.semaphore("dma_out"))
        block = ctx.enter_context(nc.Block(no_gpsimd_drain=True))

        sync_need = np.zeros(n_pieces, np.int64)
        gps_need = np.zeros(n_pieces, np.int64)
        for c, (p0, p1) in enumerate(mov_chunks_sync):
            sync_need[p0:p1] = c + 1
        for c, (p0, p1) in enumerate(mov_chunks_gps):
            gps_need[p0:p1] = c + 1
        sync_need = np.maximum.accumulate(sync_need)
        gps_need = np.maximum.accumulate(gps_need)

        @block.sync
        def _(sync):
            sync.dma_start(stat[:], stat_d[:]).then_inc(stat_sem, 16)
            for (p0, p1) in mov_chunks_sync:
                sync.dma_start(mov[:, p0 * WP:p1 * WP],
                               mov_d[:, p0 * WP:p1 * WP]).then_inc(mov_sem, 16)
            # stream drained blocks out as they are produced
            for k, (lo, hi) in enumerate(groups):
                sync.wait_ge(act_sem, k + 1)
                t = hi - lo
                sync.dma_start(
                    blk_d[:, lo * WP:hi * WP],
                    act_ring[k % 2][:, 0:t, :].rearrange("p a b -> p (a b)"),
                ).then_inc(blk_sem, 16)
            sync.wait_ge(ta_sem, n_grp)
            sync.dma_start(tmpa_d[:], tmpa[:]).then_inc(out_sem, 16)

        @block.gpsimd
        def _(gpsimd):
            for (p0, p1) in mov_chunks_gps:
                gpsimd.dma_start(
                    mov[:, p0 * WP:p1 * WP],
                    mov_d[:, p0 * WP:p1 * WP]).then_inc(mov2_sem, 16)

        @block.tensor
        def _(tensor):
            tensor.wait_ge(stat_sem, 16)
            s_seen = 0
            g_seen = 0
            for k, (lo, hi) in enumerate(groups):
                if k >= 4:
                    tensor.wait_ge(act_sem, k - 3)
                need_s = int(sync_need[hi - 1])
                need_g = int(gps_need[hi - 1])
                if need_s > s_seen:
                    tensor.wait_ge(mov_sem, 16 * need_s)
                    s_seen = need_s
                if need_g > g_seen:
                    tensor.wait_ge(mov2_sem, 16 * need_g)
                    g_seen = need_g
                base = (k % 4) * GT
                mms = group_matmuls(lo, hi)
                for mi, (plo, np_) in enumerate(mms):
                    slot = base + (plo - lo)
                    mm = nc.tensor.matmul(
                        psum[:].rearrange("p a b -> p (a b)")
                            [:, slot * WP:(slot + np_) * WP],
                        stat[:, plo * 128:(plo + 1) * 128],
                        mov[:, plo * WP:(plo + np_) * WP],
                        start=True, stop=True,
                    )
                    if mi == len(mms) - 1:
                        mm.then_inc(pe_sem, 1)

        @block.scalar
        def _(scalar):
            for k, (lo, hi) in enumerate(groups):
                scalar.wait_ge(pe_sem, k + 1)
                if k >= 2:
                    scalar.wait_ge(ta_sem, k - 1)
                    scalar.wait_ge(blk_sem, 16 * (k - 1))
                t = hi - lo
                base = (k % 4) * GT
                src = psum[:, base:base + t, :]
                dst = act_ring[k % 2][:, 0:t, :]
                nc.scalar.activation(
                    dst.rearrange("p a b -> p (a b)"),
                    src.rearrange("p a b -> p (a b)"),
                    mybir.ActivationFunctionType.Copy, scale=D2_SCALE,
                ).then_inc(act_sem, 1)

        @block.vector
        def _(vector):
            for k, (lo, hi) in enumerate(groups):
                t = hi - lo
                vector.wait_ge(act_sem, k + 1)
                buf = act_ring[k % 2]
                nc.vector.tensor_max(
                    fold[:, 0:t, :],
                    buf[:, 0:t, 0:WP // 2], buf[:, 0:t, WP // 2:WP],
                )
                nc.vector.tensor_reduce(
                    tmpa[:, lo:hi], fold[:, 0:t, :],
                    axis=X, op=MAX,
                ).then_inc(ta_sem, 1)

    return nc


def _loss_from_sums(sg, ng, sp, npnts):
    with np.errstate(divide="ignore", invalid="ignore"):
        g2p = sg / ng if ng > 0 else np.float64(np.nan)
        p2g = sp / npnts if npnts > 0 else np.float64(np.nan)
        if ng == 0 and npnts == 0:
            return np.float64(np.nan)
        ahd = (g2p + p2g) / 2.0
        return 1.0 - 1.0 / (1.0 + ahd)


RUN_OPTS = {}
LAST_RES = None
LAST_NN = None


def kernel(gth, pred):
    from concourse.bass_utils import run_bass_kernel_spmd
    import ml_dtypes

    gth = np.asarray(gth, np.float32).reshape(BC, H, W_IMG)
    pred = np.asarray(pred, np.float32).reshape(BC, H, W_IMG)

    gedge = _edge_maps(gth)
    pedge = _edge_maps(pred)

    all_jobs = []      # (npc, pair, q_ix, c_ix)
    pts = []
    for i in range(BC):
        gy, gx = np.nonzero(gedge[i])
        py, px = np.nonzero(pedge[i])
        pts.append((gy.astype(np.float32) - 128.0, gx.astype(np.float32) - 128.0,
                    py.astype(np.float32) - 128.0, px.astype(np.float32) - 128.0))
        if len(gy) and len(py):
            for q_ix, c_ix in _build_jobs_shared(gy, gx, py, px):
                npc = max(1, -(-len(c_ix) // WP))
                all_jobs.append((npc, i, q_ix, c_ix))

    order = sorted(range(len(all_jobs)),
                   key=lambda k: all_jobs[k][0], reverse=True)
    loads = [0] * N_CORES
    per_core = [[] for _ in range(N_CORES)]
    for k in order:
        c = min(range(N_CORES), key=lambda q: loads[q])
        per_core[c].append(k)
        loads[c] += all_jobs[k][0]

    for c in range(N_CORES):
        per_core[c].sort(key=lambda k: all_jobs[k][0], reverse=True)
    J = max(len(v) for v in per_core)
    slot_w = [0] * J
    for c in range(N_CORES):
        for j, k in enumerate(per_core[c]):
            slot_w[j] = max(slot_w[j], all_jobs[k][0])
    P = sum(slot_w)
    piece_job = np.zeros(P, np.int64)
    slot_off = []
    p = 0
    for j, w in enumerate(slot_w):
        slot_off.append(p)
        piece_job[p:p + w] = j
        p += w

    q1, q2, q3 = P // 4, P // 2, (3 * P) // 4
    mov_chunks_sync = [(0, q1), (q1, q2)]
    mov_chunks_gps = [(q2, q3), (q3, P)]

    nc = _build_program(P, piece_job, mov_chunks_sync, mov_chunks_gps)

    sent_stat = _aug_g(np.empty(0, np.float32), np.empty(0, np.float32), 128)
    sent_mov = _aug_p(np.empty(0, np.float32), np.empty(0, np.float32), WP)
    in_maps = []
    piece_map = []
    for c in range(N_CORES):
        stat = np.empty((6, 128 * P), np.float32)
        mov = np.empty((6, WP * P), np.float32)
        pmap = [None] * P
        for j in range(J):
            p = slot_off[j]
            w = slot_w[j]
            if j < len(per_core[c]):
                k = per_core[c][j]
                npc, i, q_ix, c_ix = all_jobs[k]
                gy, gx, py, px = pts[i]
                sa = _aug_g(gy[q_ix], gx[q_ix], 128)
                aug = _aug_p(py[c_ix], px[c_ix], w * WP)
                for t in range(w):
                    stat[:, (p + t) * 128:(p + t + 1) * 128] = sa
                    mov[:, (p + t) * WP:(p + t + 1) * WP] = \
                        aug[:, t * WP:(t + 1) * WP]
                    pmap[p + t] = k
            else:
                for t in range(w):
                    stat[:, (p + t) * 128:(p + t + 1) * 128] = sent_stat
                    mov[:, (p + t) * WP:(p + t + 1) * WP] = sent_mov
        piece_map.append(pmap)
        in_maps.append({
            "stat": stat.astype(ml_dtypes.bfloat16),
            "mov": mov.astype(ml_dtypes.bfloat16),
        })

    res = run_bass_kernel_spmd(nc, in_maps, list(range(N_CORES)), **RUN_OPTS)
    global LAST_RES, LAST_NN
    LAST_RES = res

    sums = np.zeros((BC, 2), np.float64)
    nn_dbg = {}
    # p->g per-pair NN accumulator over all pred points
    dp_min = [np.full(len(pts[i][2]), np.inf) for i in range(BC)]
    for c in range(N_CORES):
        tmpav = np.asarray(res.results[c]["tmpa"], np.float64)   # [128, P]
        blk = np.asarray(res.results[c]["blk"], np.float64)      # [128, P*WP]
        # group pieces by job slot
        seen = set()
        for pi, k in enumerate(piece_map[c]):
            if k is None or k in seen:
                continue
            seen.add(k)
            npc, i, q_ix, c_ix = all_jobs[k]
            j = per_core[c].index(k)
            p0 = slot_off[j]
            w = slot_w[j]
            nq = len(q_ix)
            # g->p: min over the job's piece columns
            d2 = (tmpav[:nq, p0:p0 + w] * D2_BACK).min(axis=1)
            dist = np.sqrt(np.maximum(d2, 0.0))
            sums[i, 0] += dist.sum()
            nn_dbg.setdefault((i, 0), []).append((q_ix, dist))
            # p->g: per-column max over partitions, scatter-min
            v = blk[:, p0 * WP:p0 * WP + len(c_ix)].max(axis=0)
            d2p = v * D2_BACK
            np.minimum.at(dp_min[i], c_ix, d2p)
    LAST_NN = nn_dbg

    losses = np.full(BC, np.nan, np.float64)
    for i in range(BC):
        gy = pts[i][0]
        py = pts[i][2]
        n_g, n_p = len(gy), len(py)
        if n_g == 0 and n_p == 0:
            continue
        if n_g == 0 or n_p == 0:
            losses[i] = _loss_from_sums(np.inf, max(n_g, 1),
                                        np.inf, max(n_p, 1))
        else:
            sp = np.sqrt(np.maximum(dp_min[i], 0.0)).sum()
            losses[i] = _loss_from_sums(sums[i, 0], n_g, sp, n_p)

    return np.float32(np.nanmean(losses.astype(np.float32)))


# revision 24
# speedup vs baseline: 4.0828x; 1.1350x over previous
"""Average Hausdorff loss on 8 Trainium2 NeuronCores.

Strategy (v4: one-matrix shared blocks, pruned, host dp-finish)
--------------------------------------------------------------
Host (numpy, cheap):
  * binarize + 3x3-erosion edge detection, compact edge coords per (b,c)
  * per pair: KD-split the GTH points into tiles of <=128. A cell-grid
    separable EDT gives per-point NN-distance upper bounds in both
    directions. A tile's candidate set = pred points p with
    dist(p, sub-bbox) <= max(UB_sub, UB_p), which provably contains
      - the NN pred point of every gth query in the tile, and
      - every pred point whose own NN gth point lies in the tile.
    So ONE distance block per gth tile serves both directions.
  * blocks are cut into uniform 128-candidate pieces, bin-packed over
    the 8 cores into uniform job slots (device program is piece-index
    uniform; per-core variation is data only).

Device (raw Bass, SPMD over 8 cores):
  PE  : merged matmuls [6,128]^T @ [6,<=512] -> PSUM -(d^2)/4 (exact
        bf16 via byte-split squared norms), 4 ping-pong PSUM regions.
  ACT : copies each 8-piece group to an SBUF fp16 ring (scale 2^-12).
  DVE : fp16 tensor_max fold + [128,8,64] tensor_reduce -> per-gth-
        query NN column per piece (g->p direction).
  DMA : streams every drained fp16 block back to DRAM.
Host: g->p from the NN columns; p->g by per-column max over the 128
partitions of the returned blocks, scatter-min by candidate id; sqrt,
means, nanmean.
"""

import math
import numpy as np

H = 256
W_IMG = 256
BC = 16
N_CORES = 8
TILE_Q = 128
SUB_Q = 16
CELL = 2
WP = 128              # uniform piece width (candidate cols)
GT = 8                # pieces per reduce group (1024 cols)
SENT = 16384.0
D2_SCALE = 2.0 ** -12
D2_BACK = -4.0 * 4096.0


def _edge_maps(x):
    m = x > 0.5
    p = np.pad(m, ((0, 0), (1, 1), (1, 1)), constant_values=True)
    e = np.ones_like(m)
    for dy in range(3):
        for dx in range(3):
            e &= p[:, dy:dy + H, dx:dx + W_IMG]
    return m & ~e


def _aug_g(cy, cx, n_pad):
    n = cy.shape[0]
    fy = np.full(n_pad, SENT, np.float32)
    fx = np.full(n_pad, SENT, np.float32)
    fy[:n] = cy
    fx[:n] = cx
    sq = fy * fy + fx * fx
    b1 = np.floor(sq / 256.0)
    b0 = sq - b1 * 256.0
    out = np.empty((6, n_pad), np.float32)
    out[0] = fy * 0.5
    out[1] = fx * 0.5
    out[2] = -b1
    out[3] = -b0
    out[4] = -64.0
    out[5] = -0.25
    return out


def _aug_p(cy, cx, n_pad):
    n = cy.shape[0]
    fy = np.full(n_pad, SENT, np.float32)
    fx = np.full(n_pad, SENT, np.float32)
    fy[:n] = cy
    fx[:n] = cx
    sq = fy * fy + fx * fx
    b1 = np.floor(sq / 256.0)
    b0 = sq - b1 * 256.0
    out = np.empty((6, n_pad), np.float32)
    out[0] = fy
    out[1] = fx
    out[2] = 64.0
    out[3] = 0.25
    out[4] = b1
    out[5] = b0
    return out


def _kd_tiles(ys, xs, tile):
    out = []

    def rec(ix):
        if len(ix) <= tile:
            out.append(ix)
            return
        yy, xx = ys[ix], xs[ix]
        k = yy if (yy.max() - yy.min() >= xx.max() - xx.min()) else xx
        n = len(ix)
        half = (n // 2 // tile) * tile or n // 2
        o = np.argsort(k, kind="stable")
        rec(ix[o[:half]])
        rec(ix[o[half:]])

    rec(np.arange(len(ys)))
    return out


def _cell_ub(dys, dxs):
    G = 256 // CELL
    occ = np.zeros((G, G), bool)
    occ[dys // CELL, dxs // CELL] = True
    BIG = np.int64(10 ** 9)
    ar = np.arange(G)
    d2 = (ar[:, None] - ar[None, :]) ** 2
    occf = np.where(occ, 0, BIG)
    gcol = (d2[:, :, None] + occf[None, :, :]).min(axis=1)
    D2 = (gcol[:, None, :] + d2[None, :, :]).min(axis=2)
    return np.sqrt(D2.astype(np.float64)) * CELL + math.sqrt(2.0) * CELL


def _build_jobs_shared(gy, gx, py, px):
    """Per pair: one job per gth tile; candidates serve both directions."""
    ub_g2p = _cell_ub(py, px)[gy // CELL, gx // CELL]
    ub_p2g = _cell_ub(gy, gx)[py // CELL, px // CELL]
    jobs = []
    for ix in _kd_tiles(gy, gx, TILE_Q):
        m = np.zeros(len(py), bool)
        for s in range(0, len(ix), SUB_Q):
            sx = ix[s:s + SUB_Q]
            u = ub_g2p[sx].max()
            y0, y1 = gy[sx].min(), gy[sx].max()
            x0, x1 = gx[sx].min(), gx[sx].max()
            dy = np.maximum(np.maximum(y0 - py, py - y1), 0)
            dx = np.maximum(np.maximum(x0 - px, px - x1), 0)
            dd = np.sqrt((dy * dy + dx * dx).astype(np.float64))
            m |= (dd <= u) | (dd <= ub_p2g)
        jobs.append((ix, np.nonzero(m)[0]))
    return jobs


def _build_program(n_pieces, piece_job, mov_chunks_sync, mov_chunks_gps):
    from contextlib import ExitStack
    import concourse.bass as bass
    import concourse.mybir as mybir

    f32 = mybir.dt.float32
    f16 = mybir.dt.float16
    bf16 = mybir.dt.bfloat16
    MAX = mybir.AluOpType.max
    X = mybir.AxisListType.X

    groups = []
    p = 0
    while p < n_pieces:
        t = min(GT, n_pieces - p)
        groups.append((p, p + t))
        p += t
    n_grp = len(groups)

    nc = bass.Bass()
    stat_d = nc.declare_dram_parameter("stat", [6, 128 * n_pieces], bf16,
                                       isOutput=False)
    mov_d = nc.declare_dram_parameter("mov", [6, WP * n_pieces], bf16,
                                      isOutput=False)
    tmpa_d = nc.declare_dram_parameter("tmpa", [128, n_pieces], f16,
                                       isOutput=True)
    blk_d = nc.declare_dram_parameter("blk", [128, WP * n_pieces], f16,
                                      isOutput=True)

    def group_matmuls(lo, hi):
        mms = []
        i = lo
        while i < hi:
            j = i
            while (j + 1 < hi and piece_job[j + 1] == piece_job[i]
                   and (j + 1 - lo) % 4 != 0):
                j += 1
            mms.append((i, j - i + 1))
            i = j + 1
        return mms

    with ExitStack() as ctx:
        stat = ctx.enter_context(
            nc.sbuf_tensor("stat_s", [6, 128 * n_pieces], bf16))
        mov = ctx.enter_context(
            nc.sbuf_tensor("mov_s", [6, WP * n_pieces], bf16))
        tmpa = ctx.enter_context(
            nc.sbuf_tensor("tmpa_s", [128, n_pieces], f16))
        act_ring = [ctx.enter_context(
            nc.sbuf_tensor(f"actr{i}", [128, GT, WP], f16)) for i in range(2)]
        fold = ctx.enter_context(
            nc.sbuf_tensor("fold_s", [128, GT, WP // 2], f16))
        psum = ctx.enter_context(
            nc.psum_tensor("ps", [128, 4096 // WP, WP], f32))

        stat_sem = ctx.enter_context(nc.semaphore("stat_in"))
        mov_sem = ctx.enter_context(nc.semaphore("mov_in"))
        mov2_sem = ctx.enter_context(nc.semaphore("mov2_in"))
        pe_sem = ctx.enter_context(nc.semaphore("pe_done"))
        act_sem = ctx.enter_context(nc.semaphore("act_done"))
        ta_sem = ctx.enter_context(nc.semaphore("tailA"))
        blk_sem = ctx.enter_context(nc.semaphore("blk_out"))
        out_sem = ctx.enter_context(nc.semaphore("dma_out"))
        block = ctx.enter_context(nc.Block(no_gpsimd_drain=True))

        sync_need = np.zeros(n_pieces, np.int64)
        for c, (p0, p1) in enumerate(mov_chunks_sync + mov_chunks_gps):
            sync_need[p0:p1] = c + 1
        sync_need = np.maximum.accumulate(sync_need)

        @block.sync
        def _(sync):
            sync.dma_start(stat[:], stat_d[:]).then_inc(stat_sem, 16)
            for (p0, p1) in mov_chunks_sync + mov_chunks_gps:
                sync.dma_start(mov[:, p0 * WP:p1 * WP],
                               mov_d[:, p0 * WP:p1 * WP]).then_inc(mov_sem, 16)
            # stream drained blocks out as they are produced
            for k, (lo, hi) in enumerate(groups):
                sync.wait_ge(act_sem, k + 1)
                t = hi - lo
                sync.dma_start(
                    blk_d[:, lo * WP:hi * WP],
                    act_ring[k % 2][:, 0:t, :].rearrange("p a b -> p (a b)"),
                ).then_inc(blk_sem, 16)
            sync.wait_ge(ta_sem, n_grp)
            sync.dma_start(tmpa_d[:], tmpa[:]).then_inc(out_sem, 16)



        @block.tensor
        def _(tensor):
            tensor.wait_ge(stat_sem, 16)
            s_seen = 0
            for k, (lo, hi) in enumerate(groups):
                if k >= 4:
                    tensor.wait_ge(act_sem, k - 3)
                need_s = int(sync_need[hi - 1])
                if need_s > s_seen:
                    tensor.wait_ge(mov_sem, 16 * need_s)
                    s_seen = need_s
                base = (k % 4) * GT
                mms = group_matmuls(lo, hi)
                for mi, (plo, np_) in enumerate(mms):
                    slot = base + (plo - lo)
                    mm = nc.tensor.matmul(
                        psum[:].rearrange("p a b -> p (a b)")
                            [:, slot * WP:(slot + np_) * WP],
                        stat[:, plo * 128:(plo + 1) * 128],
                        mov[:, plo * WP:(plo + np_) * WP],
                        start=True, stop=True,
                    )
                    if mi == len(mms) - 1:
                        mm.then_inc(pe_sem, 1)

        @block.scalar
        def _(scalar):
            for k, (lo, hi) in enumerate(groups):
                scalar.wait_ge(pe_sem, k + 1)
                if k >= 2:
                    scalar.wait_ge(ta_sem, k - 1)
                    scalar.wait_ge(blk_sem, 16 * (k - 1))
                t = hi - lo
                base = (k % 4) * GT
                src = psum[:, base:base + t, :]
                dst = act_ring[k % 2][:, 0:t, :]
                nc.scalar.activation(
                    dst.rearrange("p a b -> p (a b)"),
                    src.rearrange("p a b -> p (a b)"),
                    mybir.ActivationFunctionType.Copy, scale=D2_SCALE,
                ).then_inc(act_sem, 1)

        @block.vector
        def _(vector):
            for k, (lo, hi) in enumerate(groups):
                t = hi - lo
                vector.wait_ge(act_sem, k + 1)
                buf = act_ring[k % 2]
                nc.vector.tensor_max(
                    fold[:, 0:t, :],
                    buf[:, 0:t, 0:WP // 2], buf[:, 0:t, WP // 2:WP],
                )
                nc.vector.tensor_reduce(
                    tmpa[:, lo:hi], fold[:, 0:t, :],
                    axis=X, op=MAX,
                ).then_inc(ta_sem, 1)

    return nc


def _loss_from_sums(sg, ng, sp, npnts):
    with np.errstate(divide="ignore", invalid="ignore"):
        g2p = sg / ng if ng > 0 else np.float64(np.nan)
        p2g = sp / npnts if npnts > 0 else np.float64(np.nan)
        if ng == 0 and npnts == 0:
            return np.float64(np.nan)
        ahd = (g2p + p2g) / 2.0
        return 1.0 - 1.0 / (1.0 + ahd)


RUN_OPTS = {}
LAST_RES = None
LAST_NN = None


def kernel(gth, pred):
    from concourse.bass_utils import run_bass_kernel_spmd
    import ml_dtypes

    gth = np.asarray(gth, np.float32).reshape(BC, H, W_IMG)
    pred = np.asarray(pred, np.float32).reshape(BC, H, W_IMG)

    gedge = _edge_maps(gth)
    pedge = _edge_maps(pred)

    all_jobs = []      # (npc, pair, q_ix, c_ix)
    pts = []
    for i in range(BC):
        gy, gx = np.nonzero(gedge[i])
        py, px = np.nonzero(pedge[i])
        pts.append((gy.astype(np.float32) - 128.0, gx.astype(np.float32) - 128.0,
                    py.astype(np.float32) - 128.0, px.astype(np.float32) - 128.0))
        if len(gy) and len(py):
            for q_ix, c_ix in _build_jobs_shared(gy, gx, py, px):
                npc = max(1, -(-len(c_ix) // WP))
                all_jobs.append((npc, i, q_ix, c_ix))

    order = sorted(range(len(all_jobs)),
                   key=lambda k: all_jobs[k][0], reverse=True)
    loads = [0] * N_CORES
    per_core = [[] for _ in range(N_CORES)]
    for k in order:
        c = min(range(N_CORES), key=lambda q: loads[q])
        per_core[c].append(k)
        loads[c] += all_jobs[k][0]

    for c in range(N_CORES):
        per_core[c].sort(key=lambda k: all_jobs[k][0], reverse=True)
    J = max(len(v) for v in per_core)
    slot_w = [0] * J
    for c in range(N_CORES):
        for j, k in enumerate(per_core[c]):
            slot_w[j] = max(slot_w[j], all_jobs[k][0])
    P = sum(slot_w)
    piece_job = np.zeros(P, np.int64)
    slot_off = []
    p = 0
    for j, w in enumerate(slot_w):
        slot_off.append(p)
        piece_job[p:p + w] = j
        p += w

    q1, q2, q3 = P // 4, P // 2, (3 * P) // 4
    mov_chunks_sync = [(0, q1), (q1, q2)]
    mov_chunks_gps = [(q2, q3), (q3, P)]

    nc = _build_program(P, piece_job, mov_chunks_sync, mov_chunks_gps)

    sent_stat = _aug_g(np.empty(0, np.float32), np.empty(0, np.float32), 128)
    sent_mov = _aug_p(np.empty(0, np.float32), np.empty(0, np.float32), WP)
    in_maps = []
    piece_map = []
    for c in range(N_CORES):
        stat = np.empty((6, 128 * P), np.float32)
        mov = np.empty((6, WP * P), np.float32)
        pmap = [None] * P
        for j in range(J):
            p = slot_off[j]
            w = slot_w[j]
            if j < len(per_core[c]):
                k = per_core[c][j]
                npc, i, q_ix, c_ix = all_jobs[k]
                gy, gx, py, px = pts[i]
                sa = _aug_g(gy[q_ix], gx[q_ix], 128)
                aug = _aug_p(py[c_ix], px[c_ix], w * WP)
                for t in range(w):
                    stat[:, (p + t) * 128:(p + t + 1) * 128] = sa
                    mov[:, (p + t) * WP:(p + t + 1) * WP] = \
                        aug[:, t * WP:(t + 1) * WP]
                    pmap[p + t] = k
            else:
                for t in range(w):
                    stat[:, (p + t) * 128:(p + t + 1) * 128] = sent_stat
                    mov[:, (p + t) * WP:(p + t + 1) * WP] = sent_mov
        piece_map.append(pmap)
        in_maps.append({
            "stat": stat.astype(ml_dtypes.bfloat16),
            "mov": mov.astype(ml_dtypes.bfloat16),
        })

    res = run_bass_kernel_spmd(nc, in_maps, list(range(N_CORES)), **RUN_OPTS)
    global LAST_RES, LAST_NN
    LAST_RES = res

    sums = np.zeros((BC, 2), np.float64)
    nn_dbg = {}
    # p->g per-pair NN accumulator over all pred points
    dp_min = [np.full(len(pts[i][2]), np.inf) for i in range(BC)]
    for c in range(N_CORES):
        tmpav = np.asarray(res.results[c]["tmpa"], np.float64)   # [128, P]
        blk = np.asarray(res.results[c]["blk"], np.float64)      # [128, P*WP]
        # group pieces by job slot
        seen = set()
        for pi, k in enumerate(piece_map[c]):
            if k is None or k in seen:
                continue
            seen.add(k)
            npc, i, q_ix, c_ix = all_jobs[k]
            j = per_core[c].index(k)
            p0 = slot_off[j]
            w = slot_w[j]
            nq = len(q_ix)
            # g->p: min over the job's piece columns
            d2 = (tmpav[:nq, p0:p0 + w] * D2_BACK).min(axis=1)
            dist = np.sqrt(np.maximum(d2, 0.0))
            sums[i, 0] += dist.sum()
            nn_dbg.setdefault((i, 0), []).append((q_ix, dist))
            # p->g: per-column max over partitions, scatter-min
            v = blk[:, p0 * WP:p0 * WP + len(c_ix)].max(axis=0)
            d2p = v * D2_BACK
            np.minimum.at(dp_min[i], c_ix, d2p)
    LAST_NN = nn_dbg

    losses = np.full(BC, np.nan, np.float64)
    for i in range(BC):
        gy = pts[i][0]
        py = pts[i][2]
        n_g, n_p = len(gy), len(py)
        if n_g == 0 and n_p == 0:
            continue
        if n_g == 0 or n_p == 0:
            losses[i] = _loss_from_sums(np.inf, max(n_g, 1),
                                        np.inf, max(n_p, 1))
        else:
            sp = np.sqrt(np.maximum(dp_min[i], 0.0)).sum()
            losses[i] = _loss_from_sums(sums[i, 0], n_g, sp, n_p)

    return np.float32(np.nanmean(losses.astype(np.float32)))


# revision 27
# speedup vs baseline: 5.3539x; 1.3113x over previous
"""Average Hausdorff loss on 8 Trainium2 NeuronCores.

Strategy (v4: one-matrix shared blocks, pruned, host dp-finish)
--------------------------------------------------------------
Host (numpy, cheap):
  * binarize + 3x3-erosion edge detection, compact edge coords per (b,c)
  * per pair: KD-split the GTH points into tiles of <=128. A cell-grid
    separable EDT gives per-point NN-distance upper bounds in both
    directions. A tile's candidate set = pred points p with
    dist(p, sub-bbox) <= max(UB_sub, UB_p), which provably contains
      - the NN pred point of every gth query in the tile, and
      - every pred point whose own NN gth point lies in the tile.
    So ONE distance block per gth tile serves both directions.
  * blocks are cut into uniform 128-candidate pieces, bin-packed over
    the 8 cores into uniform job slots (device program is piece-index
    uniform; per-core variation is data only).

Device (raw Bass, SPMD over 8 cores):
  PE  : merged matmuls [6,128]^T @ [6,<=512] -> PSUM -(d^2)/4 (exact
        bf16 via byte-split squared norms), 4 ping-pong PSUM regions.
  ACT : copies each 8-piece group to an SBUF fp16 ring (scale 2^-12).
  DVE : fp16 tensor_max fold + [128,8,64] tensor_reduce -> per-gth-
        query NN column per piece (g->p direction).
  DMA : streams every drained fp16 block back to DRAM.
Host: g->p from the NN columns; p->g by per-column max over the 128
partitions of the returned blocks, scatter-min by candidate id; sqrt,
means, nanmean.
"""

import math
import numpy as np

H = 256
W_IMG = 256
BC = 16
N_CORES = 8
TILE_Q = 128
SUB_Q = 16
CELL = 2
WP = 128              # uniform piece width (candidate cols)
GT = 8                # pieces per reduce group (1024 cols)
SENT = 16384.0
D2_SCALE = 2.0 ** -12
D2_BACK = -4.0 * 4096.0


def _edge_maps(x):
    m = x > 0.5
    p = np.pad(m, ((0, 0), (1, 1), (1, 1)), constant_values=True)
    e = np.ones_like(m)
    for dy in range(3):
        for dx in range(3):
            e &= p[:, dy:dy + H, dx:dx + W_IMG]
    return m & ~e


def _aug_g(cy, cx, n_pad):
    n = cy.shape[0]
    fy = np.full(n_pad, SENT, np.float32)
    fx = np.full(n_pad, SENT, np.float32)
    fy[:n] = cy
    fx[:n] = cx
    sq = fy * fy + fx * fx
    b1 = np.floor(sq / 256.0)
    b0 = sq - b1 * 256.0
    out = np.empty((6, n_pad), np.float32)
    out[0] = fy * 0.5
    out[1] = fx * 0.5
    out[2] = -b1
    out[3] = -b0
    out[4] = -64.0
    out[5] = -0.25
    return out


def _aug_p(cy, cx, n_pad):
    n = cy.shape[0]
    fy = np.full(n_pad, SENT, np.float32)
    fx = np.full(n_pad, SENT, np.float32)
    fy[:n] = cy
    fx[:n] = cx
    sq = fy * fy + fx * fx
    b1 = np.floor(sq / 256.0)
    b0 = sq - b1 * 256.0
    out = np.empty((6, n_pad), np.float32)
    out[0] = fy
    out[1] = fx
    out[2] = 64.0
    out[3] = 0.25
    out[4] = b1
    out[5] = b0
    return out


def _kd_tiles(ys, xs, tile):
    out = []

    def rec(ix):
        if len(ix) <= tile:
            out.append(ix)
            return
        yy, xx = ys[ix], xs[ix]
        k = yy if (yy.max() - yy.min() >= xx.max() - xx.min()) else xx
        n = len(ix)
        half = (n // 2 // tile) * tile or n // 2
        o = np.argsort(k, kind="stable")
        rec(ix[o[:half]])
        rec(ix[o[half:]])

    rec(np.arange(len(ys)))
    return out


def _cell_ub(dys, dxs):
    G = 256 // CELL
    occ = np.zeros((G, G), bool)
    occ[dys // CELL, dxs // CELL] = True
    BIG = np.int64(10 ** 9)
    ar = np.arange(G)
    d2 = (ar[:, None] - ar[None, :]) ** 2
    occf = np.where(occ, 0, BIG)
    gcol = (d2[:, :, None] + occf[None, :, :]).min(axis=1)
    D2 = (gcol[:, None, :] + d2[None, :, :]).min(axis=2)
    return np.sqrt(D2.astype(np.float64)) * CELL + math.sqrt(2.0) * CELL


def _build_jobs_shared(gy, gx, py, px):
    """Per pair: one job per gth tile; candidates serve both directions."""
    ub_g2p = _cell_ub(py, px)[gy // CELL, gx // CELL]
    ub_p2g = _cell_ub(gy, gx)[py // CELL, px // CELL]
    jobs = []
    for ix in _kd_tiles(gy, gx, TILE_Q):
        m = np.zeros(len(py), bool)
        for s in range(0, len(ix), SUB_Q):
            sx = ix[s:s + SUB_Q]
            u = ub_g2p[sx].max()
            y0, y1 = gy[sx].min(), gy[sx].max()
            x0, x1 = gx[sx].min(), gx[sx].max()
            dy = np.maximum(np.maximum(y0 - py, py - y1), 0)
            dx = np.maximum(np.maximum(x0 - px, px - x1), 0)
            dd = np.sqrt((dy * dy + dx * dx).astype(np.float64))
            m |= (dd <= u) | (dd <= ub_p2g)
        jobs.append((ix, np.nonzero(m)[0]))
    return jobs


def _build_program(n_pieces, piece_job, mov_chunks_sync, mov_chunks_gps,
                   stat_split):
    from contextlib import ExitStack
    import concourse.bass as bass
    import concourse.mybir as mybir

    f32 = mybir.dt.float32
    f16 = mybir.dt.float16
    bf16 = mybir.dt.bfloat16
    MAX = mybir.AluOpType.max
    X = mybir.AxisListType.X

    groups = []
    p = 0
    while p < n_pieces:
        # smaller groups near the end shorten the pipeline-drain tail
        t = GT if n_pieces - p >= 3 * GT else min(GT // 2, n_pieces - p)
        groups.append((p, p + t))
        p += t
    n_grp = len(groups)

    nc = bass.Bass()
    stat_d = nc.declare_dram_parameter("stat", [6, 128 * n_pieces], bf16,
                                       isOutput=False)
    mov_d = nc.declare_dram_parameter("mov", [6, WP * n_pieces], bf16,
                                      isOutput=False)
    tmpa_d = nc.declare_dram_parameter("tmpa", [128, n_pieces], f16,
                                       isOutput=True)
    blk_d = nc.declare_dram_parameter("blk", [128, WP * n_pieces], f16,
                                      isOutput=True)

    def group_matmuls(lo, hi):
        mms = []
        i = lo
        while i < hi:
            j = i
            while (j + 1 < hi and piece_job[j + 1] == piece_job[i]
                   and (j + 1 - lo) % 4 != 0):
                j += 1
            mms.append((i, j - i + 1))
            i = j + 1
        return mms

    with ExitStack() as ctx:
        stat = ctx.enter_context(
            nc.sbuf_tensor("stat_s", [6, 128 * n_pieces], bf16))
        mov = ctx.enter_context(
            nc.sbuf_tensor("mov_s", [6, WP * n_pieces], bf16))
        tmpa = ctx.enter_context(
            nc.sbuf_tensor("tmpa_s", [128, n_pieces], f16))
        act_ring = [ctx.enter_context(
            nc.sbuf_tensor(f"actr{i}", [128, GT, WP], f16)) for i in range(4)]
        fold = ctx.enter_context(
            nc.sbuf_tensor("fold_s", [128, GT, WP // 2], f16))
        psum = ctx.enter_context(
            nc.psum_tensor("ps", [128, 4096 // WP, WP], f32))

        stat_sem = ctx.enter_context(nc.semaphore("stat_in"))
        mov_sem = ctx.enter_context(nc.semaphore("mov_in"))
        mov2_sem = ctx.enter_context(nc.semaphore("mov2_in"))
        pe_sem = ctx.enter_context(nc.semaphore("pe_done"))
        act_sem = ctx.enter_context(nc.semaphore("act_done"))
        ta_sem = ctx.enter_context(nc.semaphore("tailA"))
        blk_sem = ctx.enter_context(nc.semaphore("blk_out"))
        out_sem = ctx.enter_context(nc.semaphore("dma_out"))
        block = ctx.enter_context(nc.Block(no_gpsimd_drain=True))

        sync_need = np.zeros(n_pieces, np.int64)
        for c, (p0, p1) in enumerate(mov_chunks_sync + mov_chunks_gps):
            sync_need[p0:p1] = c + 1
        sync_need = np.maximum.accumulate(sync_need)

        @block.sync
        def _(sync):
            s1 = stat_split * 128
            sync.dma_start(stat[:, 0:s1], stat_d[:, 0:s1]).then_inc(stat_sem, 16)
            first = True
            for (p0, p1) in mov_chunks_sync + mov_chunks_gps:
                sync.dma_start(mov[:, p0 * WP:p1 * WP],
                               mov_d[:, p0 * WP:p1 * WP]).then_inc(mov_sem, 16)
                if first:
                    sync.dma_start(stat[:, s1:], stat_d[:, s1:]).then_inc(
                        stat_sem, 16)
                    first = False
            # stream drained blocks out as they are produced
            for k, (lo, hi) in enumerate(groups):
                sync.wait_ge(act_sem, k + 1)
                t = hi - lo
                sync.dma_start(
                    blk_d[:, lo * WP:hi * WP],
                    act_ring[k % 4][:, 0:t, :].rearrange("p a b -> p (a b)"),
                ).then_inc(blk_sem, 16)
            sync.wait_ge(ta_sem, n_grp)
            sync.dma_start(tmpa_d[:], tmpa[:]).then_inc(out_sem, 16)



        @block.tensor
        def _(tensor):
            tensor.wait_ge(stat_sem, 16)
            s_seen = 0
            st_seen = 1
            for k, (lo, hi) in enumerate(groups):
                if k >= 4:
                    tensor.wait_ge(act_sem, k - 3)
                if hi - 1 >= stat_split and st_seen < 2:
                    tensor.wait_ge(stat_sem, 32)
                    st_seen = 2
                need_s = int(sync_need[hi - 1])
                if need_s > s_seen:
                    tensor.wait_ge(mov_sem, 16 * need_s)
                    s_seen = need_s
                base = (k % 4) * GT
                mms = group_matmuls(lo, hi)
                for mi, (plo, np_) in enumerate(mms):
                    slot = base + (plo - lo)
                    mm = nc.tensor.matmul(
                        psum[:].rearrange("p a b -> p (a b)")
                            [:, slot * WP:(slot + np_) * WP],
                        stat[:, plo * 128:(plo + 1) * 128],
                        mov[:, plo * WP:(plo + np_) * WP],
                        start=True, stop=True,
                    )
                    if mi == len(mms) - 1:
                        mm.then_inc(pe_sem, 1)

        @block.scalar
        def _(scalar):
            for k, (lo, hi) in enumerate(groups):
                scalar.wait_ge(pe_sem, k + 1)
                if k >= 4 and k % 2 == 0:
                    scalar.wait_ge(ta_sem, k - 2)
                    scalar.wait_ge(blk_sem, 16 * (k - 2))
                t = hi - lo
                base = (k % 4) * GT
                src = psum[:, base:base + t, :]
                dst = act_ring[k % 4][:, 0:t, :]
                nc.scalar.activation(
                    dst.rearrange("p a b -> p (a b)"),
                    src.rearrange("p a b -> p (a b)"),
                    mybir.ActivationFunctionType.Copy, scale=D2_SCALE,
                ).then_inc(act_sem, 1)

        @block.vector
        def _(vector):
            for k, (lo, hi) in enumerate(groups):
                t = hi - lo
                vector.wait_ge(act_sem, k + 1)
                buf = act_ring[k % 4]
                nc.vector.tensor_max(
                    fold[:, 0:t, :],
                    buf[:, 0:t, 0:WP // 2], buf[:, 0:t, WP // 2:WP],
                )
                nc.vector.tensor_reduce(
                    tmpa[:, lo:hi], fold[:, 0:t, :],
                    axis=X, op=MAX,
                ).then_inc(ta_sem, 1)

    return nc


def _loss_from_sums(sg, ng, sp, npnts):
    with np.errstate(divide="ignore", invalid="ignore"):
        g2p = sg / ng if ng > 0 else np.float64(np.nan)
        p2g = sp / npnts if npnts > 0 else np.float64(np.nan)
        if ng == 0 and npnts == 0:
            return np.float64(np.nan)
        ahd = (g2p + p2g) / 2.0
        return 1.0 - 1.0 / (1.0 + ahd)


RUN_OPTS = {}
LAST_RES = None
LAST_NN = None


def kernel(gth, pred):
    from concourse.bass_utils import run_bass_kernel_spmd
    import ml_dtypes

    gth = np.asarray(gth, np.float32).reshape(BC, H, W_IMG)
    pred = np.asarray(pred, np.float32).reshape(BC, H, W_IMG)

    gedge = _edge_maps(gth)
    pedge = _edge_maps(pred)

    all_jobs = []      # (npc, pair, q_ix, c_ix)
    pts = []
    for i in range(BC):
        gy, gx = np.nonzero(gedge[i])
        py, px = np.nonzero(pedge[i])
        pts.append((gy.astype(np.float32) - 128.0, gx.astype(np.float32) - 128.0,
                    py.astype(np.float32) - 128.0, px.astype(np.float32) - 128.0))
        if len(gy) and len(py):
            for q_ix, c_ix in _build_jobs_shared(gy, gx, py, px):
                npc = max(1, -(-len(c_ix) // WP))
                all_jobs.append((npc, i, q_ix, c_ix))

    order = sorted(range(len(all_jobs)),
                   key=lambda k: all_jobs[k][0], reverse=True)
    loads = [0] * N_CORES
    per_core = [[] for _ in range(N_CORES)]
    for k in order:
        c = min(range(N_CORES), key=lambda q: loads[q])
        per_core[c].append(k)
        loads[c] += all_jobs[k][0]

    for c in range(N_CORES):
        per_core[c].sort(key=lambda k: all_jobs[k][0], reverse=True)
    J = max(len(v) for v in per_core)
    slot_w = [0] * J
    for c in range(N_CORES):
        for j, k in enumerate(per_core[c]):
            slot_w[j] = max(slot_w[j], all_jobs[k][0])
    P = sum(slot_w)
    piece_job = np.zeros(P, np.int64)
    slot_off = []
    p = 0
    for j, w in enumerate(slot_w):
        slot_off.append(p)
        piece_job[p:p + w] = j
        p += w

    c0 = min(3 * GT, P)
    stat_split = c0
    rest = P - c0
    bnds = [0, c0] + [c0 + rest * t // 4 for t in (1, 2, 3)] + [P]
    bnds = sorted(set(bnds))
    chunks = [(bnds[t], bnds[t + 1]) for t in range(len(bnds) - 1)]
    mov_chunks_sync = chunks
    mov_chunks_gps = []

    nc = _build_program(P, piece_job, mov_chunks_sync, mov_chunks_gps,
                        stat_split)

    sent_stat = _aug_g(np.empty(0, np.float32), np.empty(0, np.float32), 128)
    sent_mov = _aug_p(np.empty(0, np.float32), np.empty(0, np.float32), WP)
    in_maps = []
    piece_map = []
    for c in range(N_CORES):
        stat = np.empty((6, 128 * P), np.float32)
        mov = np.empty((6, WP * P), np.float32)
        pmap = [None] * P
        for j in range(J):
            p = slot_off[j]
            w = slot_w[j]
            if j < len(per_core[c]):
                k = per_core[c][j]
                npc, i, q_ix, c_ix = all_jobs[k]
                gy, gx, py, px = pts[i]
                sa = _aug_g(gy[q_ix], gx[q_ix], 128)
                aug = _aug_p(py[c_ix], px[c_ix], w * WP)
                for t in range(w):
                    stat[:, (p + t) * 128:(p + t + 1) * 128] = sa
                    mov[:, (p + t) * WP:(p + t + 1) * WP] = \
                        aug[:, t * WP:(t + 1) * WP]
                    pmap[p + t] = k
            else:
                for t in range(w):
                    stat[:, (p + t) * 128:(p + t + 1) * 128] = sent_stat
                    mov[:, (p + t) * WP:(p + t + 1) * WP] = sent_mov
        piece_map.append(pmap)
        in_maps.append({
            "stat": stat.astype(ml_dtypes.bfloat16),
            "mov": mov.astype(ml_dtypes.bfloat16),
        })

    res = run_bass_kernel_spmd(nc, in_maps, list(range(N_CORES)), **RUN_OPTS)
    global LAST_RES, LAST_NN
    LAST_RES = res

    sums = np.zeros((BC, 2), np.float64)
    nn_dbg = {}
    # p->g per-pair NN accumulator over all pred points
    dp_min = [np.full(len(pts[i][2]), np.inf) for i in range(BC)]
    for c in range(N_CORES):
        tmpav = np.asarray(res.results[c]["tmpa"], np.float64)   # [128, P]
        blk = np.asarray(res.results[c]["blk"], np.float64)      # [128, P*WP]
        # group pieces by job slot
        seen = set()
        for pi, k in enumerate(piece_map[c]):
            if k is None or k in seen:
                continue
            seen.add(k)
            npc, i, q_ix, c_ix = all_jobs[k]
            j = per_core[c].index(k)
            p0 = slot_off[j]
            w = slot_w[j]
            nq = len(q_ix)
            # g->p: min over the job's piece columns
            d2 = (tmpav[:nq, p0:p0 + w] * D2_BACK).min(axis=1)
            dist = np.sqrt(np.maximum(d2, 0.0))
            sums[i, 0] += dist.sum()
            nn_dbg.setdefault((i, 0), []).append((q_ix, dist))
            # p->g: per-column max over partitions, scatter-min
            v = blk[:, p0 * WP:p0 * WP + len(c_ix)].max(axis=0)
            d2p = v * D2_BACK
            np.minimum.at(dp_min[i], c_ix, d2p)
    LAST_NN = nn_dbg

    losses = np.full(BC, np.nan, np.float64)
    for i in range(BC):
        gy = pts[i][0]
        py = pts[i][2]
        n_g, n_p = len(gy), len(py)
        if n_g == 0 and n_p == 0:
            continue
        if n_g == 0 or n_p == 0:
            losses[i] = _loss_from_sums(np.inf, max(n_g, 1),
                                        np.inf, max(n_p, 1))
        else:
            sp = np.sqrt(np.maximum(dp_min[i], 0.0)).sum()
            losses[i] = _loss_from_sums(sums[i, 0], n_g, sp, n_p)

    return np.float32(np.nanmean(losses.astype(np.float32)))
